# revision 37
# baseline (speedup 1.0000x reference)
"""Answer-pointer network forward pass on 8 TRN2 NeuronCores.

Data-parallel over batch: B=64 -> 8 batches per core, weights replicated.
No collectives; each core computes softmax attention maps (aP1, aP2) for
its batch shard and the host concatenates.

Schedule: the 8 per-core batches are split into G=4 groups of GB=2 and
software-pipelined. Nothing couples batches mathematically (softmax, ct,
GRU are all per-batch), so group g's pointer-step-2 (scalar-engine-bound:
tanh over [LP, GB, H]) runs concurrently with group g+1's pointer-step-1
(tensor-engine-bound: the WPh linear over [LP, GB, 2H]). This hides the
~37us of step-2 tanh that a batch-monolithic schedule serializes after
the GRU, leaving only the last group's step-2 exposed at the tail.

Within a group, P1 is chunk-pipelined over 4 p-chunks of 512: passP
matmuls (tensor) -> PSUM->SBUF copies (vector/gpsimd) -> tanh (scalar)
-> masked sP matmuls (tensor) -> per-chunk exp with Z-accumulation
(scalar). sP/exp for chunk pc are emitted one iteration later than the
chunk's tanh so no engine queue ever heads into a not-yet-satisfied
dependency (engines execute their queues in order; a stalled head op
blocks ready work behind it).

Layouts (host-side prep, outside HW exec):
  - peC  [pc, b, 128, kt, 512] fp16: passEnc feature-major for the WPh
    linear (contract over d=512 on partitions).
  - peN8 [pr, 128, b, 2, 512] fp8e4: passEnc position-major for the
    attention-weighted context ct, pre-packed for fp8 DoubleRow matmuls
    (contract over p=2048; each DR instruction consumes K=256).

Measured PE cost law: a matmul instruction costs ~N_out_columns x
0.417ns at full p-state; the tensor engine clock ramps with sustained
use (idle gaps drop it to ~0.83ns/col for up to 3us), so the schedule
aims to keep the PE continuously fed.

The GRU consumes the *unnormalized* context sum (matmul is linear) so
its matmuls start before the softmax normalizer 1/Z is ready; 1/Z is
folded into the gate bias-add. Sigmoid is computed as 0.5*tanh(x/2)+0.5
so every activation in the kernel (tanh/exp/identity) lives in the one
'exp_and_others' table -- zero ACT_TABLE_LOAD switches after the first.

Per-batch reductions (sP, sQ, rQ, ct) use masked stationary operands:
column (b mod GB) of the lhsT kept, rest zeroed, so batch b's matmul
writes only PSUM row (b mod GB); accumulating over the group assembles
[GB, N] without partition-offset copies.
"""

import numpy as np

try:
    import concourse.bass as bass
except ImportError:  # pragma: no cover
    import sys

    sys.path.insert(0, "/opt/trn_rl_repo")
    import concourse.bass as bass

import concourse.tile as tile
from concourse import bacc, mybir
from concourse.bass_utils import run_bass_kernel_spmd

F8 = mybir.dt.float8e4
F16 = mybir.dt.float16
F32 = mybir.dt.float32
AF = mybir.ActivationFunctionType
OP = mybir.AluOpType
DR = mybir.MatmulPerfMode.DoubleRow

H = 256      # hidden
D = 512      # 2*hidden
LP = 2048    # passage length
LQ = 64      # question length
B = 64       # global batch
BL = 8       # batch per core
G6 = 6 * H   # 1536, GRU gate width
NC = 8       # cores
NKT = D // 128    # 4 contraction tiles over d
NHT = H // 128    # 2 tiles over h
NPC = LP // 512   # 4 p-chunks of 512
NPR = LP // 256   # 8 p-pairs of 256 (DoubleRow K tiles)
NG = 4            # batch groups per core
GB = BL // NG     # 2 batches per group


def _layout(entries):
    off, table = 0, {}
    for name, ln in entries:
        table[name] = (off, ln)
        off += ln
    return table, off


WA, WATOT = _layout([
    ("WQvT", NHT * H), ("WQuT", NKT * H), ("WPhT", NKT * H), ("WahT", NKT * H),
    ("VQrT", NHT), ("vt1m", NHT * BL * BL), ("vt2g", NHT * BL * GB),
    ("idh", 128), ("colm", BL * BL), ("cm16", BL * 16),
])
WQ, WQTOT = _layout([("qeT", NKT * BL * LQ)])
# GRU weight matrices in fp8 (inputs rq1/ct are small weighted averages;
# quantization noise lands ~5e-4 on the final softmax)
WB, WBTOT = _layout([("wihT", NKT * G6), ("whhT", NKT * G6)])
W32, W32TOT = _layout([("idf", 128), ("cqb", NHT), ("wb", NHT)])

_CACHED_NC = None


def _build():
    nc = bacc.Bacc("TRN2", target_bir_lowering=False, debug=False, num_devices=NC)

    peC = nc.dram_tensor("peC", [NG, NPC, 128, GB, NKT, 512], F16,
                         kind="ExternalInput").ap()
    peN8 = nc.dram_tensor("peN8", [NG, NPR // 2, 128, 2, GB, 2, 512], F8,
                         kind="ExternalInput").ap()
    wpA = nc.dram_tensor("wpA", [128, WATOT], F16, kind="ExternalInput").ap()
    wpQ = nc.dram_tensor("wpQ", [128, WQTOT], F16, kind="ExternalInput").ap()
    wpN = nc.dram_tensor("wpN", [LQ, BL * D], F16, kind="ExternalInput").ap()
    wpG = nc.dram_tensor("wpG", [128, 12 * (GB + BL)], F16,
                         kind="ExternalInput").ap()
    wpB = nc.dram_tensor("wpB", [128, WBTOT], F8, kind="ExternalInput").ap()
    wp32 = nc.dram_tensor("wp32", [128, W32TOT], F32, kind="ExternalInput").ap()
    out = nc.dram_tensor("out", [2, BL, LP], F32, kind="ExternalOutput").ap()

    with tile.TileContext(nc) as tc:
        sing = tc.alloc_tile_pool(name="sing", bufs=1)

        def _single(shape, dtype, name):
            return sing.tile(shape, dtype, name=name, tag=name)

        chunkp = tc.alloc_tile_pool(name="chunk", bufs=2)
        pnp = tc.alloc_tile_pool(name="pn", bufs=4)
        t2p = tc.alloc_tile_pool(name="t2", bufs=3)
        t2bp = tc.alloc_tile_pool(name="t2b", bufs=3)
        apb = tc.alloc_tile_pool(name="apb", bufs=2)
        wmp = tc.alloc_tile_pool(name="wm", bufs=2)
        grup = tc.alloc_tile_pool(name="gru", bufs=1)
        # PSUM budget: ppp 3 banks + rowps 3 + trp 1 + ctps 1 = 8
        ppp = tc.alloc_tile_pool(name="ppp", bufs=3, space="PSUM")
        rowps = tc.alloc_tile_pool(name="rowps", bufs=3, space="PSUM")
        trp = tc.alloc_tile_pool(name="trp", bufs=1, space="PSUM")
        ctps = tc.alloc_tile_pool(name="ctps", bufs=1, space="PSUM")

        # ---- packed weights, hot-first ----
        wpA_s = _single([128, WATOT], F16, "wpA_s")
        nc.sync.dma_start(wpA_s, wpA)
        pe_tiles = {}
        pn_tiles = {}

        def fetch_peC(c):
            g, pc = divmod(c, NPC)
            t = chunkp.tile([128, GB, NKT, 512], F16, tag="pe", name=f"pe{c}")
            eng = nc.sync if c % 2 == 0 else nc.gpsimd
            eng.dma_start(t, peC[g, pc])
            pe_tiles[(g, pc)] = t

        def fetch_pn(g, j):
            t = pnp.tile([128, 2, GB, 2, 512], F8, tag="pn", name=f"pn{g}_{j}")
            nc.sync.dma_start(t, peN8[g, j])
            pn_tiles[(g, j)] = t

        wpQ_s = _single([128, WQTOT], F16, "wpQ_s")
        nc.sync.dma_start(wpQ_s, wpQ)
        wp32_s = _single([128, W32TOT], F32, "wp32_s")
        nc.sync.dma_start(wp32_s, wp32)
        fetch_peC(0)
        wpN_s = _single([LQ, BL * D], F16, "wpN_s")
        nc.gpsimd.dma_start(wpN_s, wpN)
        fetch_peC(1)
        wpB_s = _single([128, WBTOT], F8, "wpB_s")
        nc.gpsimd.dma_start(wpB_s, wpB)
        wpG_s = _single([128, 12 * (GB + BL)], F16, "wpG_s")
        nc.gpsimd.dma_start(wpG_s, wpG)

        def sA(name):
            o, ln = WA[name]
            return wpA_s[:, o:o + ln]

        WQvT_s = sA("WQvT").rearrange("p (kt h) -> p kt h", kt=NHT)
        WQuT_s = sA("WQuT").rearrange("p (kt h) -> p kt h", kt=NKT)
        WPhT_s = sA("WPhT").rearrange("p (kt h) -> p kt h", kt=NKT)
        WahT_s = sA("WahT").rearrange("p (kt h) -> p kt h", kt=NKT)
        VQrT_s = sA("VQrT").rearrange("p (ht o) -> p ht o", ht=NHT)
        vt1m_s = sA("vt1m").rearrange("p (ht b c) -> p ht b c", ht=NHT, b=BL)
        vt2g_s = sA("vt2g").rearrange("p (ht b c) -> p ht b c", ht=NHT, b=BL)
        idh_s = sA("idh")
        colm_s = sA("colm").rearrange("p (b c) -> p b c", b=BL)
        cm16_s = sA("cm16").rearrange("p (b m) -> p b m", b=BL)
        qeT_s = wpQ_s[:, WQ["qeT"][0]:WQ["qeT"][0] + NKT * BL * LQ].rearrange(
            "p (kt bq) -> p kt bq", kt=NKT)
        qeN_s = wpN_s[:, :]
        wihT_s = wpB_s[:, WB["wihT"][0]:WB["wihT"][0] + NKT * G6].rearrange(
            "p (pr sub g) -> p pr sub g", pr=NKT // 2, sub=2)
        whhT_s = wpB_s[:, WB["whhT"][0]:WB["whhT"][0] + NKT * G6].rearrange(
            "p (pr sub g) -> p pr sub g", pr=NKT // 2, sub=2)
        bhhT_s = wpG_s[:, 0:12 * GB].rearrange("p (gt c) -> p gt c", gt=12)
        bihT_s = wpG_s[:, 12 * GB:12 * (GB + BL)].rearrange(
            "p (gt c) -> p gt c", gt=12)
        idf_s = wp32_s[:, W32["idf"][0]:W32["idf"][0] + 128]
        cqb_s = wp32_s[:, W32["cqb"][0]:W32["cqb"][0] + NHT]
        wb_s = wp32_s[:, W32["wb"][0]:W32["wb"][0] + NHT].rearrange(
            "p (ht o) -> p ht o", ht=NHT)

        # persistent activations
        ppr_s = _single([128, NHT, BL, NPC, 512], F16, "ppr_s")  # raw passP
        biasP_s = _single([128, 2, NHT, BL], F32, "biasP_s")
        # per-group exp(sP) rows at base partition 0 (matmul/transpose
        # inputs must start at partition 0/32/64)
        w1p = tc.alloc_tile_pool(name="w1p", bufs=2)
        w2p = tc.alloc_tile_pool(name="w2p", bufs=2)
        zlp = tc.alloc_tile_pool(name="zlp", bufs=2)
        w1_t, w2_t = {}, {}
        # engine accesses must start at partition 0 (mult-of-32), so all
        # per-group [GB, ...] data lives in its own base-0 tile
        zpart_t, zp2_t, z1_t, rz1_t, z2_t, rz2_t = ({} for _ in range(6))
        rq1_s = _single([BL, D], F32, "rq1_s")
        rq1T_s = [_single([128, BL], F16, f"rq1T{k}") for k in range(NKT)]
        rq1T8_s = [_single([128, 2, 16], F8, f"rq1T8{k}") for k in range(NKT // 2)]
        giT_s = _single([128, 12, BL], F16, "giT_s")
        colm8_s = _single([128, BL, 16], F8, "colm8_s")
        nc.vector.tensor_copy(colm8_s, cm16_s)

        def bcast_dim(ap, axis, size):
            """Insert a stride-0 (broadcast) free dim at position axis."""
            entries = list(ap.ap)
            entries.insert(axis, [0, size])
            return bass.AP(tensor=ap.tensor, offset=ap.offset, ap=entries)

        # ================= pipelined P1 / P2 stages =================
        t1saved, t2saved, sp1_ps, sp2_ps = {}, {}, {}, {}

        def em_a(g, pc):
            """passP matmuls for group g, chunk pc; copies PSUM -> ppr."""
            pe = pe_tiles.pop((g, pc))
            for b in range(GB):
                pps = [ppp.tile([128, 512], F32, tag="acc", name=f"pp{ht}")
                       for ht in range(NHT)]
                for kt in range(NKT):
                    for ht in range(NHT):
                        nc.tensor.matmul(pps[ht],
                                         lhsT=WPhT_s[:, kt, ht * 128:(ht + 1) * 128],
                                         rhs=pe[:, b, kt, :],
                                         start=kt == 0, stop=kt == NKT - 1)
                for ht in range(NHT):
                    nc.vector.tensor_copy(ppr_s[:, ht, GB * g + b, pc, :],
                                          pps[ht])

        def em_b_tanh(g, pc):
            for b in range(GB):
                t2 = t2p.tile([128, NHT, 512], F16, tag="t2", name="t2")
                for ht in range(NHT):
                    nc.scalar.activation(t2[:, ht, :],
                                         ppr_s[:, ht, GB * g + b, pc, :], AF.Tanh,
                                         bias=biasP_s[:, 0, ht,
                                                      GB * g + b:GB * g + b + 1],
                                         scale=1.0)
                t1saved[(g, pc, b)] = t2

        def em_sp1(g, pc):
            ps = rowps.tile([GB, 512], F32, tag="row", name=f"sp1_{pc}")
            for b in range(GB):
                t2 = t1saved.pop((g, pc, b))
                for ht in range(NHT):
                    nc.tensor.matmul(ps, lhsT=vt2g_s[:, ht, GB * g + b, :],
                                     rhs=t2[:, ht, :],
                                     start=(b == 0 and ht == 0),
                                     stop=(b == GB - 1 and ht == NHT - 1))
            sp1_ps[(g, pc)] = ps

        def em_exp1(g, pc):
            ps = sp1_ps.pop((g, pc))
            if pc == 0:
                w1_t[g] = w1p.tile([GB, LP], F16, tag="w1", name=f"w1_{g}")
                zpart_t[g] = zlp.tile([GB, NPC], F32, tag="zp1", name=f"zp1_{g}")
            nc.scalar.activation(w1_t[g][:, pc * 512:(pc + 1) * 512], ps, AF.Exp,
                                 accum_out=zpart_t[g][:, pc:pc + 1])

        def em_p2_tanh(g, pc):
            for b in range(GB):
                t2 = t2bp.tile([128, NHT, 512], F16, tag="t2b", name="t2b")
                for ht in range(NHT):
                    nc.scalar.activation(t2[:, ht, :],
                                         ppr_s[:, ht, GB * g + b, pc, :], AF.Tanh,
                                         bias=biasP_s[:, 1, ht,
                                                      GB * g + b:GB * g + b + 1],
                                         scale=1.0)
                t2saved[(g, pc, b)] = t2

        def em_p2_mm(g, pc):
            ps = rowps.tile([GB, 512], F32, tag="row", name=f"sp2_{pc}")
            for b in range(GB):
                t2 = t2saved.pop((g, pc, b))
                for ht in range(NHT):
                    nc.tensor.matmul(ps, lhsT=vt2g_s[:, ht, GB * g + b, :],
                                     rhs=t2[:, ht, :],
                                     start=(b == 0 and ht == 0),
                                     stop=(b == GB - 1 and ht == NHT - 1))
            sp2_ps[(g, pc)] = ps

        def em_p2_exp(g, pc):
            ps = sp2_ps.pop((g, pc))
            if pc == 0:
                w2_t[g] = w2p.tile([GB, LP], F16, tag="w2", name=f"w2_{g}")
                zp2_t[g] = zlp.tile([GB, NPC], F32, tag="zp2", name=f"zp2_{g}")
            nc.scalar.activation(w2_t[g][:, pc * 512:(pc + 1) * 512], ps, AF.Exp,
                                 accum_out=zp2_t[g][:, pc:pc + 1])

        def finish_p2(g):
            rows = slice(GB * g, GB * (g + 1))
            z2 = zlp.tile([GB, 1], F32, tag="z2", name=f"z2_{g}")
            rz2 = zlp.tile([GB, 1], F32, tag="rz2", name=f"rz2_{g}")
            nc.vector.reduce_sum(z2, zp2_t.pop(g), axis=mybir.AxisListType.X)
            nc.vector.reciprocal(rz2, z2)
            w2g = w2_t.pop(g)
            for h in range(2):
                hs = slice(h * LP // 2, (h + 1) * LP // 2)
                apc = apb.tile([GB, LP // 2], F32, tag="ap", name="ap2c")
                nc.vector.tensor_scalar(apc, w2g[:, hs], rz2, None,
                                        op0=OP.mult)
                nc.sync.dma_start(out=out[1, rows, hs], in_=apc)

        # ================= group epilogue: softmax Z, ct, GRU =============
        def gru(g, ps_ct):
            """GRU cell in transposed layout: gates live as [128 gate-rows,
            12 g-tiles, GB b-cols] so every elementwise op has free size
            <= 24 (elementwise cost is free-size-bound, partition-count
            free). 1/Z is folded into the ct PSUM->SBUF copy (per-partition
            scale in the [GB, D] layout), so everything downstream uses
            normalized ct and no per-column scaling is ever needed."""
            rows = slice(GB * g, GB * (g + 1))   # free-dim index into biasP
            ctn = grup.tile([GB, D], F16, tag="ctn", name=f"ctn{g}")
            nc.vector.tensor_scalar(ctn, ps_ct, rz1_t[g], None, op0=OP.mult)
            ctT16 = grup.tile([128, NKT, GB], F16, tag="ctT16", name=f"ctT16{g}")
            ctT8 = [grup.tile([128, 2, 16], F8, tag=f"ctT8{j}", name=f"ctT8_{g}{j}")
                    for j in range(NKT // 2)]
            for kt in range(NKT):
                ps_t = trp.tile([128, GB], F16, tag="tr", name="ps_t")
                nc.tensor.transpose(ps_t, ctn[:, kt * 128:(kt + 1) * 128],
                                    idh_s[:GB, :GB])
                nc.vector.tensor_copy(ctT16[:, kt, :], ps_t)
                nc.vector.tensor_copy(ctT8[kt // 2][:, kt % 2, 0:GB], ps_t)
            # ghT[gate, b] = (ct_norm @ whh.T).T / 16 + bhh  (weights are x16)
            ghT = grup.tile([128, 12, GB], F16, tag="ghT", name=f"ghT{g}")
            for gt in range(12):
                ps_g = rowps.tile([128, GB], F32, tag="row", name="ps_g")
                for p2 in range(NKT // 2):
                    nc.tensor.matmul(
                        ps_g, lhsT=whhT_s[:, p2, :, gt * 128:(gt + 1) * 128],
                        rhs=ctT8[p2][:, :, 0:GB],
                        start=p2 == 0, stop=p2 == NKT // 2 - 1, perf_mode=DR)
                nc.vector.tensor_scalar(ghT[:, gt, :], ps_g, 1.0 / 16.0, None,
                                        op0=OP.mult)
            nc.vector.tensor_add(ghT, ghT, bhhT_s)
            # r,z = sigmoid(gi+gh) as 0.5*tanh(x/2)+0.5 (stays in the
            # exp/tanh activation table -> no ACT_TABLE_LOAD switches);
            # gate rows: g-tiles 0:4 = r, 4:8 = z, 8:12 = n
            giTg = giT_s[:, :, rows]
            rzin = grup.tile([128, 8, GB], F16, tag="rzin", name="rzin")
            nc.vector.tensor_add(rzin, giTg[:, 0:8, :], ghT[:, 0:8, :])
            th = grup.tile([128, 8, GB], F16, tag="th", name="th")
            nc.scalar.activation(th, rzin, AF.Tanh, scale=0.5)
            # n = tanh(gi_n + r*gh_n); r*gh_n = 0.5*(th_r*gh_n + gh_n)
            a_t = grup.tile([128, 4, GB], F16, tag="ga", name="ga")
            nc.vector.tensor_mul(a_t, th[:, 0:4, :], ghT[:, 8:12, :])
            nc.vector.tensor_add(a_t, a_t, ghT[:, 8:12, :])
            nin = grup.tile([128, 4, GB], F16, tag="nin", name="nin")
            nc.vector.scalar_tensor_tensor(nin, a_t, 0.5, giTg[:, 8:12, :],
                                           op0=OP.mult, op1=OP.add)
            n_t = grup.tile([128, 4, GB], F16, tag="gn", name="gn")
            nc.scalar.activation(n_t, nin, AF.Tanh)
            # h' = n + z*(ct-n); z*(ct-n) = 0.5*(th_z*d + d), d = ctT - n
            d_t = grup.tile([128, 4, GB], F16, tag="gd", name="gd")
            nc.vector.tensor_sub(d_t, ctT16, n_t)
            e_t = grup.tile([128, 4, GB], F16, tag="ge", name="ge")
            nc.vector.tensor_mul(e_t, th[:, 4:8, :], d_t)
            nc.vector.tensor_add(e_t, e_t, d_t)
            rq2T = grup.tile([128, 4, GB], F16, tag="rq2T", name="rq2T")
            nc.vector.scalar_tensor_tensor(rq2T, e_t, 0.5, n_t,
                                           op0=OP.mult, op1=OP.add)
            for ht in range(NHT):
                ps_w = trp.tile([128, GB], F32, tag="tr", name="ps_w")
                for kt in range(NKT):
                    nc.tensor.matmul(ps_w,
                                     lhsT=WahT_s[:, kt, ht * 128:(ht + 1) * 128],
                                     rhs=rq2T[:, kt, :], start=kt == 0,
                                     stop=kt == NKT - 1)
                nc.vector.tensor_scalar(biasP_s[:, 1, ht, rows], ps_w,
                                        wb_s[:, ht, :], None, op0=OP.add)

        def group_end(g):
            rows = slice(GB * g, GB * (g + 1))
            # 1/Z for step 1: ready while the ct matmuls run
            z1_t[g] = zlp.tile([GB, 1], F32, tag="z1", name=f"z1_{g}")
            rz1_t[g] = zlp.tile([GB, 1], F32, tag="rz1", name=f"rz1_{g}")
            nc.vector.reduce_sum(z1_t[g], zpart_t.pop(g),
                                 axis=mybir.AxisListType.X)
            nc.vector.reciprocal(rz1_t[g], z1_t[g])
            # ct += w1[rows] @ passEnc, via fp8 DoubleRow
            ps_ct = ctps.tile([GB, D], F32, tag="ct", name=f"ct{g}")
            for j in range(NPR // 2):
                pn = pn_tiles.pop((g, j))
                for i in range(2):
                    pr = 2 * j + i
                    wm8 = wmp.tile([128, 2, GB, 16], F8, tag="wm", name="wm8")
                    for sub in range(2):
                        pt = pr * 2 + sub
                        ps_wt = trp.tile([128, GB], F16, tag="tr", name="ps_wt")
                        nc.tensor.transpose(
                            ps_wt, w1_t[g][:, pt * 128:(pt + 1) * 128],
                            idh_s[:GB, :GB])
                        nc.vector.tensor_mul(wm8[:, sub],
                                             bcast_dim(ps_wt[:, :], 2, 16),
                                             colm8_s[:, 0:GB, :])
                    for b in range(GB):
                        nc.tensor.matmul(
                            ps_ct, lhsT=wm8[:, :, b, 0:GB], rhs=pn[:, i, b, :, :],
                            start=(pr == 0 and b == 0),
                            stop=(pr == NPR - 1 and b == GB - 1),
                            perf_mode=DR)
            if g + 1 < NG:
                # hoist the next group's first passP block so the tensor
                # queue has ready work while the GRU serial chain resolves
                em_a(g + 1, 0)
            gru(g, ps_ct)
            # aP1 normalize + store: off the critical path
            w1g = w1_t.pop(g)
            for h in range(2):
                hs = slice(h * LP // 2, (h + 1) * LP // 2)
                apc = apb.tile([GB, LP // 2], F32, tag="ap", name="ap1c")
                nc.vector.tensor_scalar(apc, w1g[:, hs], rz1_t[g], None,
                                        op0=OP.mult)
                nc.gpsimd.dma_start(out=out[0, rows, hs], in_=apc)

        # ---- Q phase: question-aware initial state rQ, all 8 batches ----
        tqT_s = _single([128, NHT, BL * LQ], F16, "tqT_s")
        cb_s = _single([128, NHT], F32, "cb_s")
        esq = _single([BL, LQ], F32, "esq")
        zq = _single([BL, 1], F32, "zq")
        rzq = _single([BL, 1], F32, "rzq")
        a_s = _single([BL, LQ], F16, "a_s")
        atm_s = _single([LQ, BL, BL], F16, "atm_s")

        def q1():
            ps_qv = trp.tile([128, NHT], F32, tag="tr", name="ps_qv")
            for ht in range(NHT):
                for kt in range(NHT):
                    nc.tensor.matmul(ps_qv[:, ht:ht + 1],
                                     lhsT=WQvT_s[:, kt, ht * 128:(ht + 1) * 128],
                                     rhs=VQrT_s[:, kt, :], start=kt == 0,
                                     stop=kt == NHT - 1)
            nc.vector.tensor_add(cb_s, ps_qv, cqb_s)
            for ht in range(NHT):
                ps_tq = ppp.tile([128, 512], F32, tag="acc", name="ps_tq")
                for kt in range(NKT):
                    nc.tensor.matmul(ps_tq,
                                     lhsT=WQuT_s[:, kt, ht * 128:(ht + 1) * 128],
                                     rhs=qeT_s[:, kt, :], start=kt == 0,
                                     stop=kt == NKT - 1)
                nc.scalar.activation(tqT_s[:, ht, :], ps_tq, AF.Tanh,
                                     bias=cb_s[:, ht:ht + 1], scale=1.0)

        def q2():
            ps_sq = rowps.tile([BL, LQ], F32, tag="row", name="ps_sq")
            for b in range(BL):
                for ht in range(NHT):
                    nc.tensor.matmul(ps_sq, lhsT=vt1m_s[:, ht, b, :],
                                     rhs=tqT_s[:, ht, b * LQ:(b + 1) * LQ],
                                     start=(b == 0 and ht == 0),
                                     stop=(b == BL - 1 and ht == NHT - 1))
            nc.scalar.activation(esq, ps_sq, AF.Exp, accum_out=zq)
            nc.vector.reciprocal(rzq, zq)
            nc.vector.tensor_scalar(a_s, esq, rzq, None, op0=OP.mult)

        def q3():
            ps_at = trp.tile([LQ, BL], F16, tag="tr", name="ps_at")
            nc.tensor.transpose(ps_at, a_s, idh_s[:BL, :BL])
            nc.vector.tensor_mul(atm_s,
                                 bcast_dim(ps_at[:, :], 1, BL),
                                 colm_s[0:LQ, :, :])
            ps_rq = rowps.tile([BL, D], F32, tag="row", name="ps_rq")
            for b in range(BL):
                nc.tensor.matmul(ps_rq, lhsT=atm_s[:, b, :],
                                 rhs=qeN_s[:, b * D:(b + 1) * D],
                                 start=b == 0, stop=b == BL - 1)
            nc.vector.tensor_copy(rq1_s, ps_rq)

        def q4():
            for kt in range(NKT):
                ps_t = trp.tile([128, BL], F32, tag="tr", name="ps_q4")
                nc.tensor.transpose(ps_t, rq1_s[:, kt * 128:(kt + 1) * 128],
                                    idf_s[:BL, :BL])
                nc.vector.tensor_copy(rq1T_s[kt], ps_t)
                nc.vector.tensor_copy(rq1T8_s[kt // 2][:, kt % 2, 0:BL], ps_t)
            for ht in range(NHT):
                ps_w = trp.tile([128, BL], F32, tag="tr", name="ps_w0")
                for kt in range(NKT):
                    nc.tensor.matmul(ps_w,
                                     lhsT=WahT_s[:, kt, ht * 128:(ht + 1) * 128],
                                     rhs=rq1T_s[kt], start=kt == 0,
                                     stop=kt == NKT - 1)
                nc.vector.tensor_scalar(biasP_s[:, 0, ht, :], ps_w,
                                        wb_s[:, ht, :], None, op0=OP.add)

        def emit_gi():
            # giT[gate, b] = (rq1 @ wih.T).T / 16 + bih, fp8 DoubleRow,
            # all 8 batches at once in transposed layout
            for gt in range(12):
                ps_gi = rowps.tile([128, BL], F32, tag="row", name="ps_gi")
                for pr in range(NKT // 2):
                    nc.tensor.matmul(
                        ps_gi, lhsT=wihT_s[:, pr, :, gt * 128:(gt + 1) * 128],
                        rhs=rq1T8_s[pr][:, :, 0:BL],
                        start=pr == 0, stop=pr == NKT // 2 - 1, perf_mode=DR)
                nc.vector.tensor_scalar(giT_s[:, gt, :], ps_gi, 1.0 / 16.0,
                                        None, op0=OP.mult)
            nc.vector.tensor_add(giT_s, giT_s, bihT_s)

        # ================= emission =================
        q1()
        fetch_peC(2)
        fetch_pn(0, 0)
        em_a(0, 0)
        q2()
        fetch_peC(3)
        fetch_pn(0, 1)
        em_a(0, 1)
        q3()
        fetch_peC(4)
        fetch_pn(0, 2)
        em_a(0, 2)
        q4()
        em_b_tanh(0, 0)
        fetch_peC(5)
        fetch_pn(0, 3)
        em_a(0, 3)
        em_sp1(0, 0)
        em_exp1(0, 0)
        em_b_tanh(0, 1)
        em_sp1(0, 1)
        em_exp1(0, 1)
        em_b_tanh(0, 2)
        em_sp1(0, 2)
        em_exp1(0, 2)
        em_b_tanh(0, 3)
        em_sp1(0, 3)
        em_exp1(0, 3)
        emit_gi()
        group_end(0)

        for g in range(1, NG):
            for pc in range(NPC):
                c = NPC * g + pc
                if c + 2 < NG * NPC:
                    fetch_peC(c + 2)
                fetch_pn(g, pc)
                if pc == 0:
                    # em_a(g, 0) was hoisted into group_end(g-1); tanh1
                    # first here (its deps are older than tanh2's, which
                    # waits on gru(g-1))
                    em_b_tanh(g, 0)
                    em_p2_tanh(g - 1, 0)
                    em_p2_mm(g - 1, 0)
                    em_p2_exp(g - 1, 0)
                else:
                    em_sp1(g, pc - 1)
                    em_exp1(g, pc - 1)
                    em_a(g, pc)
                    em_p2_tanh(g - 1, pc)
                    em_p2_mm(g - 1, pc)
                    em_b_tanh(g, pc)
                    em_p2_exp(g - 1, pc)
            em_sp1(g, NPC - 1)
            em_exp1(g, NPC - 1)
            finish_p2(g - 1)
            group_end(g)

        # tail: last group's pointer step 2
        for pc in range(NPC):
            em_p2_tanh(NG - 1, pc)
            em_p2_mm(NG - 1, pc)
            em_p2_exp(NG - 1, pc)
        finish_p2(NG - 1)

        zlp.release()
        w2p.release()
        w1p.release()
        ctps.release()
        trp.release()
        rowps.release()
        ppp.release()
        grup.release()
        wmp.release()
        apb.release()
        t2bp.release()
        t2p.release()
        pnp.release()
        chunkp.release()
        sing.release()

    nc.compile()
    return nc


def _get_nc():
    global _CACHED_NC
    if _CACHED_NC is None:
        _CACHED_NC = _build()
    return _CACHED_NC


def _tiles(mat, nkt):  # [nkt*128, X] -> [128, nkt*X]
    x = mat.shape[1]
    return np.ascontiguousarray(
        mat.reshape(nkt, 128, x).transpose(1, 0, 2).reshape(128, nkt * x))


def _packA(f, Vt1, Vt2):
    # Vt1, Vt2: [BL, H] for this core's batch slice
    wp = np.zeros((128, WATOT), dtype=np.float16)

    def put(name, arr):
        o, ln = WA[name]
        assert arr.shape[1] == ln, (name, arr.shape, ln)
        wp[:arr.shape[0], o:o + ln] = arr

    put("WQvT", _tiles(f["WQv_W"].T.astype(np.float16), NHT))
    put("WQuT", _tiles(f["WQu_W"].T.astype(np.float16), NKT))
    put("WPhT", _tiles(f["WPh_W"].T.astype(np.float16), NKT))
    put("WahT", _tiles(f["Wah_W"].T.astype(np.float16), NKT))
    put("VQrT", _tiles(f["VQr"].reshape(1, H).T.astype(np.float16), NHT))
    # vt1m [128, ht, b, col]: col b = Vt1[b] per ht tile, rest zero
    v1 = np.zeros((128, NHT, BL, BL), dtype=np.float16)
    for b in range(BL):
        v1[:, :, b, b] = Vt1[b].reshape(NHT, 128).T
    put("vt1m", v1.reshape(128, NHT * BL * BL))
    # vt2g [128, ht, b, col]: col (b % GB) = Vt2[b], rest zero
    v2 = np.zeros((128, NHT, BL, GB), dtype=np.float16)
    for b in range(BL):
        v2[:, :, b, b % GB] = Vt2[b].reshape(NHT, 128).T
    put("vt2g", v2.reshape(128, NHT * BL * GB))
    put("idh", np.eye(128, dtype=np.float16))
    put("colm", np.broadcast_to(np.eye(BL, dtype=np.float16).reshape(1, BL * BL),
                                (128, BL * BL)))
    cm16 = np.hstack([np.eye(BL, dtype=np.float16),
                      np.zeros((BL, 16 - BL), dtype=np.float16)])
    put("cm16", np.broadcast_to(cm16.reshape(1, BL * 16), (128, BL * 16)))
    return wp


def _packG(f):
    # transposed gate biases, broadcast along the b (free) axis:
    # bhhT [128, 12, GB] then bihT [128, 12, BL]
    wp = np.zeros((128, 12 * (GB + BL)), dtype=np.float16)
    bhh = f["gru_bhh"].astype(np.float16).reshape(12, 128).T
    bih = f["gru_bih"].astype(np.float16).reshape(12, 128).T
    wp[:, 0:12 * GB] = np.repeat(bhh[:, :, None], GB, axis=2).reshape(128, -1)
    wp[:, 12 * GB:] = np.repeat(bih[:, :, None], BL, axis=2).reshape(128, -1)
    return wp


def _packQ(qe):
    wp = np.zeros((128, WQTOT), dtype=np.float16)
    o, ln = WQ["qeT"]
    qeT = np.ascontiguousarray(qe.transpose(2, 1, 0)).astype(np.float16)
    wp[:, o:o + ln] = _tiles(qeT.reshape(D, BL * LQ), NKT)
    return wp


def _packB(f):
    # x16 lifts the ~N(0, 0.05^2) weights out of fp8's subnormal range;
    # compensated on-chip (gi: x1/16 in the bias add; gh: cancels the
    # ct fp8 copy's 1/16 pre-scale)
    wp = np.zeros((128, WBTOT), dtype=np.float32)
    o, ln = WB["wihT"]
    wp[:, o:o + ln] = _tiles(f["gru_wih"].T.astype(np.float32) * 16.0, NKT)
    o, ln = WB["whhT"]
    wp[:, o:o + ln] = _tiles(f["gru_whh"].T.astype(np.float32) * 16.0, NKT)
    return _fp8(wp)


def _pack32(f):
    wp = np.zeros((128, W32TOT), dtype=np.float32)
    o, ln = W32["idf"]
    wp[:, o:o + ln] = np.eye(128, dtype=np.float32)
    o, ln = W32["cqb"]
    wp[:, o:o + ln] = (f["WQu_b"] + f["WQv_b"]).astype(np.float32).reshape(NHT, 128).T
    o, ln = W32["wb"]
    wp[:, o:o + ln] = (f["WPh_b"] + f["Wah_b"]).astype(np.float32).reshape(NHT, 128).T
    return wp


def _fp8(x):
    import ml_dtypes
    return np.ascontiguousarray(x).astype(ml_dtypes.float8_e4m3).view(np.uint8)


def make_in_maps(f):
    passEnc, quesEnc = f["passEnc"], f["quesEnc"]
    wp32 = _pack32(f)
    wpB = _packB(f)
    in_maps = []
    for i in range(NC):
        s = slice(i * BL, (i + 1) * BL)
        pe = passEnc[:, s, :]
        qe = quesEnc[:, s, :]
        wpA_ = _packA(f, f["Vt1"][s, :, 0], f["Vt2"][s, :, 0])
        wpQ_ = _packQ(qe)
        # peC [g, pc, part, b', kt, d]: per-partition runs of 8KB
        peC = np.ascontiguousarray(
            pe.astype(np.float16).reshape(NPC, 512, NG, GB, NKT, 128).transpose(
                2, 0, 5, 3, 4, 1))
        # peN8 [g, j, part, i, b', sub, d]: global p = (2j+i)*256 + sub*128
        # + part; per-partition contiguous runs of 4KB
        peN8 = _fp8(pe.reshape(NPR // 2, 2, 2, 128, NG, GB, D).transpose(
            4, 0, 3, 1, 5, 2, 6))
        in_maps.append({
            "peC": peC,
            "peN8": peN8,
            "wpA": wpA_, "wpQ": wpQ_, "wpB": wpB, "wp32": wp32,
            "wpN": qe.astype(np.float16).reshape(LQ, BL * D),
            "wpG": _packG(f),
        })
    return in_maps


def kernel(**inputs):
    f = {k: np.asarray(v) for k, v in inputs.items()}
    in_maps = make_in_maps(f)
    nc = _get_nc()
    res = run_bass_kernel_spmd(nc, in_maps, core_ids=list(range(NC)))
    aP1 = np.concatenate([res.results[i]["out"][0] for i in range(NC)], axis=0)
    aP2 = np.concatenate([res.results[i]["out"][1] for i in range(NC)], axis=0)
    return (aP1.astype(np.float32), aP2.astype(np.float32))


# revision 40
# speedup vs baseline: 1.0808x; 1.0808x over previous
"""Answer-pointer network forward pass on 8 TRN2 NeuronCores.

Data-parallel over batch: B=64 -> 8 batches per core, weights replicated.
No collectives; each core computes softmax attention maps (aP1, aP2) for
its batch shard and the host concatenates.

Schedule: the 8 per-core batches are split into G=4 groups of GB=2 and
software-pipelined. Nothing couples batches mathematically (softmax, ct,
GRU are all per-batch), so group g's pointer-step-2 (scalar-engine-bound:
tanh over [LP, GB, H]) runs concurrently with group g+1's pointer-step-1
(tensor-engine-bound: the WPh linear over [LP, GB, 2H]). This hides the
~37us of step-2 tanh that a batch-monolithic schedule serializes after
the GRU, leaving only the last group's step-2 exposed at the tail.

Within a group, P1 is chunk-pipelined over 4 p-chunks of 512: passP
matmuls (tensor) -> PSUM->SBUF copies (vector/gpsimd) -> tanh (scalar)
-> masked sP matmuls (tensor) -> per-chunk exp with Z-accumulation
(scalar). sP/exp for chunk pc are emitted one iteration later than the
chunk's tanh so no engine queue ever heads into a not-yet-satisfied
dependency (engines execute their queues in order; a stalled head op
blocks ready work behind it).

Layouts (host-side prep, outside HW exec):
  - peC  [pc, b, 128, kt, 512] fp16: passEnc feature-major for the WPh
    linear (contract over d=512 on partitions).
  - peN8 [pr, 128, b, 2, 512] fp8e4: passEnc position-major for the
    attention-weighted context ct, pre-packed for fp8 DoubleRow matmuls
    (contract over p=2048; each DR instruction consumes K=256).

Measured PE cost law: a matmul instruction costs ~N_out_columns x
0.417ns at full p-state; the tensor engine clock ramps with sustained
use (idle gaps drop it to ~0.83ns/col for up to 3us), so the schedule
aims to keep the PE continuously fed.

The GRU consumes the *unnormalized* context sum (matmul is linear) so
its matmuls start before the softmax normalizer 1/Z is ready; 1/Z is
folded into the gate bias-add. Sigmoid is computed as 0.5*tanh(x/2)+0.5
so every activation in the kernel (tanh/exp/identity) lives in the one
'exp_and_others' table -- zero ACT_TABLE_LOAD switches after the first.

Per-batch reductions (sP, sQ, rQ, ct) use masked stationary operands:
column (b mod GB) of the lhsT kept, rest zeroed, so batch b's matmul
writes only PSUM row (b mod GB); accumulating over the group assembles
[GB, N] without partition-offset copies.
"""

import numpy as np

try:
    import concourse.bass as bass
except ImportError:  # pragma: no cover
    import sys

    sys.path.insert(0, "/opt/trn_rl_repo")
    import concourse.bass as bass

import concourse.tile as tile
from concourse import bacc, mybir
from concourse.bass_utils import run_bass_kernel_spmd

F8 = mybir.dt.float8e4
F16 = mybir.dt.float16
F32 = mybir.dt.float32
AF = mybir.ActivationFunctionType
OP = mybir.AluOpType
DR = mybir.MatmulPerfMode.DoubleRow

H = 256      # hidden
D = 512      # 2*hidden
LP = 2048    # passage length
LQ = 64      # question length
B = 64       # global batch
BL = 8       # batch per core
G6 = 6 * H   # 1536, GRU gate width
NC = 8       # cores
NKT = D // 128    # 4 contraction tiles over d
NHT = H // 128    # 2 tiles over h
NPC = LP // 512   # 4 p-chunks of 512
NPR = LP // 256   # 8 p-pairs of 256 (DoubleRow K tiles)
NG = 4            # batch groups per core
GB = BL // NG     # 2 batches per group


def _layout(entries):
    off, table = 0, {}
    for name, ln in entries:
        table[name] = (off, ln)
        off += ln
    return table, off


WA, WATOT = _layout([
    ("WQvT", NHT * H), ("WQuT", NKT * H), ("WPhT", NKT * H), ("WahT", NKT * H),
    ("VQrT", NHT), ("vt1m", NHT * BL * BL), ("vt2g", NHT * BL * GB),
    ("idh", 128), ("colm", BL * BL), ("cm16", BL * 16),
])
WQ, WQTOT = _layout([("qeT", NKT * BL * LQ)])
# GRU weight matrices in fp8 (inputs rq1/ct are small weighted averages;
# quantization noise lands ~5e-4 on the final softmax)
WB, WBTOT = _layout([("wihT", NKT * G6), ("whhT", NKT * G6)])
W32, W32TOT = _layout([("idf", 128), ("cqb", NHT), ("wb", NHT)])

_CACHED_NC = None


def _build():
    nc = bacc.Bacc("TRN2", target_bir_lowering=False, debug=False, num_devices=NC)

    peC = nc.dram_tensor("peC", [NG, NPC, 128, GB, NKT, 512], F16,
                         kind="ExternalInput").ap()
    peN8 = nc.dram_tensor("peN8", [NG, NPR // 2, 128, 2, GB, 2, 512], F8,
                         kind="ExternalInput").ap()
    wpA = nc.dram_tensor("wpA", [128, WATOT], F16, kind="ExternalInput").ap()
    wpQ = nc.dram_tensor("wpQ", [128, WQTOT], F16, kind="ExternalInput").ap()
    wpN = nc.dram_tensor("wpN", [LQ, BL * D], F16, kind="ExternalInput").ap()
    wpG = nc.dram_tensor("wpG", [128, 12 * (GB + BL)], F16,
                         kind="ExternalInput").ap()
    wpB = nc.dram_tensor("wpB", [128, WBTOT], F8, kind="ExternalInput").ap()
    wp32 = nc.dram_tensor("wp32", [128, W32TOT], F32, kind="ExternalInput").ap()
    out = nc.dram_tensor("out", [2, BL, LP], F32, kind="ExternalOutput").ap()

    with tile.TileContext(nc) as tc:
        sing = tc.alloc_tile_pool(name="sing", bufs=1)

        def _single(shape, dtype, name):
            return sing.tile(shape, dtype, name=name, tag=name)

        chunkp = tc.alloc_tile_pool(name="chunk", bufs=3)
        pnp = tc.alloc_tile_pool(name="pn", bufs=4)
        t2p = tc.alloc_tile_pool(name="t2", bufs=3)
        t2bp = tc.alloc_tile_pool(name="t2b", bufs=3)
        apb = tc.alloc_tile_pool(name="apb", bufs=2)
        wmp = tc.alloc_tile_pool(name="wm", bufs=2)
        grup = tc.alloc_tile_pool(name="gru", bufs=1)
        # PSUM budget: ppp 3 banks + rowps 3 + trp 1 + ctps 1 = 8
        ppp = tc.alloc_tile_pool(name="ppp", bufs=3, space="PSUM")
        rowps = tc.alloc_tile_pool(name="rowps", bufs=3, space="PSUM")
        trp = tc.alloc_tile_pool(name="trp", bufs=1, space="PSUM")
        ctps = tc.alloc_tile_pool(name="ctps", bufs=1, space="PSUM")

        # ---- packed weights, hot-first ----
        wpA_s = _single([128, WATOT], F16, "wpA_s")
        nc.sync.dma_start(wpA_s, wpA)
        pe_tiles = {}
        pn_tiles = {}

        def fetch_peC(c):
            g, pc = divmod(c, NPC)
            t = chunkp.tile([128, GB, NKT, 512], F16, tag="pe", name=f"pe{c}")
            eng = nc.sync if c % 2 == 0 else nc.gpsimd
            eng.dma_start(t, peC[g, pc])
            pe_tiles[(g, pc)] = t

        def fetch_pn(g, j):
            t = pnp.tile([128, 2, GB, 2, 512], F8, tag="pn", name=f"pn{g}_{j}")
            nc.sync.dma_start(t, peN8[g, j])
            pn_tiles[(g, j)] = t

        wpQ_s = _single([128, WQTOT], F16, "wpQ_s")
        nc.sync.dma_start(wpQ_s, wpQ)
        wp32_s = _single([128, W32TOT], F32, "wp32_s")
        nc.sync.dma_start(wp32_s, wp32)
        fetch_peC(0)
        wpN_s = _single([LQ, BL * D], F16, "wpN_s")
        nc.gpsimd.dma_start(wpN_s, wpN)
        fetch_peC(1)
        wpB_s = _single([128, WBTOT], F8, "wpB_s")
        nc.gpsimd.dma_start(wpB_s, wpB)
        wpG_s = _single([128, 12 * (GB + BL)], F16, "wpG_s")
        nc.gpsimd.dma_start(wpG_s, wpG)

        def sA(name):
            o, ln = WA[name]
            return wpA_s[:, o:o + ln]

        WQvT_s = sA("WQvT").rearrange("p (kt h) -> p kt h", kt=NHT)
        WQuT_s = sA("WQuT").rearrange("p (kt h) -> p kt h", kt=NKT)
        WPhT_s = sA("WPhT").rearrange("p (kt h) -> p kt h", kt=NKT)
        WahT_s = sA("WahT").rearrange("p (kt h) -> p kt h", kt=NKT)
        VQrT_s = sA("VQrT").rearrange("p (ht o) -> p ht o", ht=NHT)
        vt1m_s = sA("vt1m").rearrange("p (ht b c) -> p ht b c", ht=NHT, b=BL)
        vt2g_s = sA("vt2g").rearrange("p (ht b c) -> p ht b c", ht=NHT, b=BL)
        idh_s = sA("idh")
        colm_s = sA("colm").rearrange("p (b c) -> p b c", b=BL)
        cm16_s = sA("cm16").rearrange("p (b m) -> p b m", b=BL)
        qeT_s = wpQ_s[:, WQ["qeT"][0]:WQ["qeT"][0] + NKT * BL * LQ].rearrange(
            "p (kt bq) -> p kt bq", kt=NKT)
        qeN_s = wpN_s[:, :]
        wihT_s = wpB_s[:, WB["wihT"][0]:WB["wihT"][0] + NKT * G6].rearrange(
            "p (pr sub g) -> p pr sub g", pr=NKT // 2, sub=2)
        whhT_s = wpB_s[:, WB["whhT"][0]:WB["whhT"][0] + NKT * G6].rearrange(
            "p (pr sub g) -> p pr sub g", pr=NKT // 2, sub=2)
        bhhT_s = wpG_s[:, 0:12 * GB].rearrange("p (gt c) -> p gt c", gt=12)
        bihT_s = wpG_s[:, 12 * GB:12 * (GB + BL)].rearrange(
            "p (gt c) -> p gt c", gt=12)
        idf_s = wp32_s[:, W32["idf"][0]:W32["idf"][0] + 128]
        cqb_s = wp32_s[:, W32["cqb"][0]:W32["cqb"][0] + NHT]
        wb_s = wp32_s[:, W32["wb"][0]:W32["wb"][0] + NHT].rearrange(
            "p (ht o) -> p ht o", ht=NHT)

        # persistent activations
        ppr_s = _single([128, NHT, BL, NPC, 512], F16, "ppr_s")  # raw passP
        biasP_s = _single([128, 2, NHT, BL], F32, "biasP_s")
        # per-group exp(sP) rows at base partition 0 (matmul/transpose
        # inputs must start at partition 0/32/64)
        w1p = tc.alloc_tile_pool(name="w1p", bufs=2)
        w2p = tc.alloc_tile_pool(name="w2p", bufs=2)
        zlp = tc.alloc_tile_pool(name="zlp", bufs=2)
        w1_t, w2_t = {}, {}
        # engine accesses must start at partition 0 (mult-of-32), so all
        # per-group [GB, ...] data lives in its own base-0 tile
        zpart_t, zp2_t, z1_t, rz1_t, z2_t, rz2_t = ({} for _ in range(6))
        rq1_s = _single([BL, D], F32, "rq1_s")
        rq1T_s = [_single([128, BL], F16, f"rq1T{k}") for k in range(NKT)]
        rq1T8_s = [_single([128, 2, 16], F8, f"rq1T8{k}") for k in range(NKT // 2)]
        giT_s = _single([128, 12, BL], F16, "giT_s")
        colm8_s = _single([128, BL, 16], F8, "colm8_s")
        nc.vector.tensor_copy(colm8_s, cm16_s)

        def bcast_dim(ap, axis, size):
            """Insert a stride-0 (broadcast) free dim at position axis."""
            entries = list(ap.ap)
            entries.insert(axis, [0, size])
            return bass.AP(tensor=ap.tensor, offset=ap.offset, ap=entries)

        # ================= pipelined P1 / P2 stages =================
        t1saved, t2saved, sp1_ps, sp2_ps = {}, {}, {}, {}

        def em_a(g, pc):
            """passP matmuls for group g, chunk pc; copies PSUM -> ppr."""
            pe = pe_tiles.pop((g, pc))
            for b in range(GB):
                pps = [ppp.tile([128, 512], F32, tag="acc", name=f"pp{ht}")
                       for ht in range(NHT)]
                for kt in range(NKT):
                    for ht in range(NHT):
                        nc.tensor.matmul(pps[ht],
                                         lhsT=WPhT_s[:, kt, ht * 128:(ht + 1) * 128],
                                         rhs=pe[:, b, kt, :],
                                         start=kt == 0, stop=kt == NKT - 1)
                for ht in range(NHT):
                    nc.vector.tensor_copy(ppr_s[:, ht, GB * g + b, pc, :],
                                          pps[ht])

        def em_b_tanh(g, pc):
            for b in range(GB):
                t2 = t2p.tile([128, NHT, 512], F16, tag="t2", name="t2")
                for ht in range(NHT):
                    nc.scalar.activation(t2[:, ht, :],
                                         ppr_s[:, ht, GB * g + b, pc, :], AF.Tanh,
                                         bias=biasP_s[:, 0, ht,
                                                      GB * g + b:GB * g + b + 1],
                                         scale=1.0)
                t1saved[(g, pc, b)] = t2

        def em_sp1(g, pc):
            ps = rowps.tile([GB, 512], F32, tag="row", name=f"sp1_{pc}")
            for b in range(GB):
                t2 = t1saved.pop((g, pc, b))
                for ht in range(NHT):
                    nc.tensor.matmul(ps, lhsT=vt2g_s[:, ht, GB * g + b, :],
                                     rhs=t2[:, ht, :],
                                     start=(b == 0 and ht == 0),
                                     stop=(b == GB - 1 and ht == NHT - 1))
            sp1_ps[(g, pc)] = ps

        def em_exp1(g, pc):
            ps = sp1_ps.pop((g, pc))
            if pc == 0:
                w1_t[g] = w1p.tile([GB, LP], F16, tag="w1", name=f"w1_{g}")
                zpart_t[g] = zlp.tile([GB, NPC], F32, tag="zp1", name=f"zp1_{g}")
            nc.scalar.activation(w1_t[g][:, pc * 512:(pc + 1) * 512], ps, AF.Exp,
                                 accum_out=zpart_t[g][:, pc:pc + 1])

        def em_p2_tanh(g, pc):
            for b in range(GB):
                t2 = t2bp.tile([128, NHT, 512], F16, tag="t2b", name="t2b")
                for ht in range(NHT):
                    nc.scalar.activation(t2[:, ht, :],
                                         ppr_s[:, ht, GB * g + b, pc, :], AF.Tanh,
                                         bias=biasP_s[:, 1, ht,
                                                      GB * g + b:GB * g + b + 1],
                                         scale=1.0)
                t2saved[(g, pc, b)] = t2

        def em_p2_mm(g, pc):
            ps = rowps.tile([GB, 512], F32, tag="row", name=f"sp2_{pc}")
            for b in range(GB):
                t2 = t2saved.pop((g, pc, b))
                for ht in range(NHT):
                    nc.tensor.matmul(ps, lhsT=vt2g_s[:, ht, GB * g + b, :],
                                     rhs=t2[:, ht, :],
                                     start=(b == 0 and ht == 0),
                                     stop=(b == GB - 1 and ht == NHT - 1))
            sp2_ps[(g, pc)] = ps

        def em_p2_exp(g, pc):
            ps = sp2_ps.pop((g, pc))
            if pc == 0:
                w2_t[g] = w2p.tile([GB, LP], F16, tag="w2", name=f"w2_{g}")
                zp2_t[g] = zlp.tile([GB, NPC], F32, tag="zp2", name=f"zp2_{g}")
            nc.scalar.activation(w2_t[g][:, pc * 512:(pc + 1) * 512], ps, AF.Exp,
                                 accum_out=zp2_t[g][:, pc:pc + 1])

        def finish_p2(g):
            rows = slice(GB * g, GB * (g + 1))
            z2 = zlp.tile([GB, 1], F32, tag="z2", name=f"z2_{g}")
            rz2 = zlp.tile([GB, 1], F32, tag="rz2", name=f"rz2_{g}")
            nc.vector.reduce_sum(z2, zp2_t.pop(g), axis=mybir.AxisListType.X)
            nc.vector.reciprocal(rz2, z2)
            w2g = w2_t.pop(g)
            for h in range(2):
                hs = slice(h * LP // 2, (h + 1) * LP // 2)
                apc = apb.tile([GB, LP // 2], F32, tag="ap", name="ap2c")
                nc.vector.tensor_scalar(apc, w2g[:, hs], rz2, None,
                                        op0=OP.mult)
                nc.sync.dma_start(out=out[1, rows, hs], in_=apc)

        # ================= group epilogue: softmax Z, ct, GRU =============
        grust = {}

        def gru_a(g, ps_ct):
            """ct normalize + transpose + ghT matmuls. 1/Z is folded into
            the ct PSUM->SBUF copy (per-partition scale in [GB, D] layout),
            so everything downstream uses normalized ct."""
            ctn = grup.tile([GB, D], F16, tag="ctn", name=f"ctn{g}")
            nc.vector.tensor_scalar(ctn, ps_ct, rz1_t[g], None, op0=OP.mult)
            ctT16 = grup.tile([128, NKT, GB], F16, tag="ctT16", name=f"ctT16{g}")
            ctT8 = [grup.tile([128, 2, 16], F8, tag=f"ctT8{j}", name=f"ctT8_{g}{j}")
                    for j in range(NKT // 2)]
            for kt in range(NKT):
                ps_t = trp.tile([128, GB], F16, tag="tr", name="ps_t")
                nc.tensor.transpose(ps_t, ctn[:, kt * 128:(kt + 1) * 128],
                                    idh_s[:GB, :GB])
                nc.vector.tensor_copy(ctT16[:, kt, :], ps_t)
                nc.vector.tensor_copy(ctT8[kt // 2][:, kt % 2, 0:GB], ps_t)
            # ghT[gate, b] = (ct_norm @ whh.T).T / 16 + bhh  (weights are x16)
            ghT = grup.tile([128, 12, GB], F16, tag="ghT", name=f"ghT{g}")
            for gt in range(12):
                ps_g = rowps.tile([128, GB], F32, tag="row", name="ps_g")
                for p2 in range(NKT // 2):
                    nc.tensor.matmul(
                        ps_g, lhsT=whhT_s[:, p2, :, gt * 128:(gt + 1) * 128],
                        rhs=ctT8[p2][:, :, 0:GB],
                        start=p2 == 0, stop=p2 == NKT // 2 - 1, perf_mode=DR)
                nc.vector.tensor_scalar(ghT[:, gt, :], ps_g, 1.0 / 16.0, None,
                                        op0=OP.mult)
            nc.vector.tensor_add(ghT, ghT, bhhT_s)
            grust[g] = (ctT16, ghT)

        def gru_b(g):
            """gate elementwise chain in transposed layout ([128, <=12, GB]:
            free size <= 24 per op). r,z = sigmoid(gi+gh) computed as
            0.5*tanh(x/2)+0.5 (stays in the exp/tanh activation table ->
            no ACT_TABLE_LOAD switches). Gate rows: 0:4 = r, 4:8 = z,
            8:12 = n."""
            ctT16, ghT = grust[g]
            giTg = giT_s[:, :, slice(GB * g, GB * (g + 1))]
            rzin = grup.tile([128, 8, GB], F16, tag="rzin", name="rzin")
            nc.vector.tensor_add(rzin, giTg[:, 0:8, :], ghT[:, 0:8, :])
            th = grup.tile([128, 8, GB], F16, tag="th", name="th")
            nc.scalar.activation(th, rzin, AF.Tanh, scale=0.5)
            # n = tanh(gi_n + r*gh_n); r*gh_n = 0.5*(th_r*gh_n + gh_n)
            a_t = grup.tile([128, 4, GB], F16, tag="ga", name="ga")
            nc.vector.tensor_mul(a_t, th[:, 0:4, :], ghT[:, 8:12, :])
            nc.vector.tensor_add(a_t, a_t, ghT[:, 8:12, :])
            nin = grup.tile([128, 4, GB], F16, tag="nin", name="nin")
            nc.vector.scalar_tensor_tensor(nin, a_t, 0.5, giTg[:, 8:12, :],
                                           op0=OP.mult, op1=OP.add)
            n_t = grup.tile([128, 4, GB], F16, tag="gn", name="gn")
            nc.scalar.activation(n_t, nin, AF.Tanh)
            # h' = n + z*(ct-n); z*(ct-n) = 0.5*(th_z*d + d), d = ctT - n
            d_t = grup.tile([128, 4, GB], F16, tag="gd", name="gd")
            nc.vector.tensor_sub(d_t, ctT16, n_t)
            e_t = grup.tile([128, 4, GB], F16, tag="ge", name="ge")
            nc.vector.tensor_mul(e_t, th[:, 4:8, :], d_t)
            nc.vector.tensor_add(e_t, e_t, d_t)
            rq2T = grup.tile([128, 4, GB], F16, tag="rq2T", name="rq2T")
            nc.vector.scalar_tensor_tensor(rq2T, e_t, 0.5, n_t,
                                           op0=OP.mult, op1=OP.add)
            grust[g] = rq2T

        def gru_c(g):
            rq2T = grust.pop(g)
            rows = slice(GB * g, GB * (g + 1))
            for ht in range(NHT):
                ps_w = trp.tile([128, GB], F32, tag="tr", name="ps_w")
                for kt in range(NKT):
                    nc.tensor.matmul(ps_w,
                                     lhsT=WahT_s[:, kt, ht * 128:(ht + 1) * 128],
                                     rhs=rq2T[:, kt, :], start=kt == 0,
                                     stop=kt == NKT - 1)
                nc.vector.tensor_scalar(biasP_s[:, 1, ht, rows], ps_w,
                                        wb_s[:, ht, :], None, op0=OP.add)

        def group_end(g):
            rows = slice(GB * g, GB * (g + 1))
            if g + 1 < NG:
                fetch_peC(NPC * (g + 1) + 2)
            # 1/Z for step 1: ready while the ct matmuls run
            z1_t[g] = zlp.tile([GB, 1], F32, tag="z1", name=f"z1_{g}")
            rz1_t[g] = zlp.tile([GB, 1], F32, tag="rz1", name=f"rz1_{g}")
            nc.vector.reduce_sum(z1_t[g], zpart_t.pop(g),
                                 axis=mybir.AxisListType.X)
            nc.vector.reciprocal(rz1_t[g], z1_t[g])
            # ct += w1[rows] @ passEnc, via fp8 DoubleRow
            ps_ct = ctps.tile([GB, D], F32, tag="ct", name=f"ct{g}")
            for j in range(NPR // 2):
                pn = pn_tiles.pop((g, j))
                for i in range(2):
                    pr = 2 * j + i
                    wm8 = wmp.tile([128, 2, GB, 16], F8, tag="wm", name="wm8")
                    for sub in range(2):
                        pt = pr * 2 + sub
                        ps_wt = trp.tile([128, GB], F16, tag="tr", name="ps_wt")
                        nc.tensor.transpose(
                            ps_wt, w1_t[g][:, pt * 128:(pt + 1) * 128],
                            idh_s[:GB, :GB])
                        nc.vector.tensor_mul(wm8[:, sub],
                                             bcast_dim(ps_wt[:, :], 2, 16),
                                             colm8_s[:, 0:GB, :])
                    for b in range(GB):
                        nc.tensor.matmul(
                            ps_ct, lhsT=wm8[:, :, b, 0:GB], rhs=pn[:, i, b, :, :],
                            start=(pr == 0 and b == 0),
                            stop=(pr == NPR - 1 and b == GB - 1),
                            perf_mode=DR)
            # the GRU serial chain is interleaved with the next group's
            # first two passP blocks so the tensor queue never runs dry
            if g + 1 < NG:
                fetch_pn(g + 1, 0)
                em_a(g + 1, 0)
                em_b_tanh(g + 1, 0)
            gru_a(g, ps_ct)
            if g + 1 < NG:
                em_sp1(g + 1, 0)
                em_exp1(g + 1, 0)
            gru_b(g)
            if g + 1 < NG:
                fetch_peC(NPC * (g + 1) + 3)
                em_a(g + 1, 1)
                em_b_tanh(g + 1, 1)
            gru_c(g)
            # aP1 normalize + store: off the critical path
            w1g = w1_t.pop(g)
            for h in range(2):
                hs = slice(h * LP // 2, (h + 1) * LP // 2)
                apc = apb.tile([GB, LP // 2], F32, tag="ap", name="ap1c")
                nc.vector.tensor_scalar(apc, w1g[:, hs], rz1_t[g], None,
                                        op0=OP.mult)
                nc.gpsimd.dma_start(out=out[0, rows, hs], in_=apc)

        # ---- Q phase        # ---- Q phase: question-aware initial state rQ, all 8 batches ----
        tqT_s = _single([128, NHT, BL * LQ], F16, "tqT_s")
        cb_s = _single([128, NHT], F32, "cb_s")
        esq = _single([BL, LQ], F32, "esq")
        zq = _single([BL, 1], F32, "zq")
        rzq = _single([BL, 1], F32, "rzq")
        a_s = _single([BL, LQ], F16, "a_s")
        atm_s = _single([LQ, BL, BL], F16, "atm_s")

        def q1():
            ps_qv = trp.tile([128, NHT], F32, tag="tr", name="ps_qv")
            for ht in range(NHT):
                for kt in range(NHT):
                    nc.tensor.matmul(ps_qv[:, ht:ht + 1],
                                     lhsT=WQvT_s[:, kt, ht * 128:(ht + 1) * 128],
                                     rhs=VQrT_s[:, kt, :], start=kt == 0,
                                     stop=kt == NHT - 1)
            nc.vector.tensor_add(cb_s, ps_qv, cqb_s)
            for ht in range(NHT):
                ps_tq = ppp.tile([128, 512], F32, tag="acc", name="ps_tq")
                for kt in range(NKT):
                    nc.tensor.matmul(ps_tq,
                                     lhsT=WQuT_s[:, kt, ht * 128:(ht + 1) * 128],
                                     rhs=qeT_s[:, kt, :], start=kt == 0,
                                     stop=kt == NKT - 1)
                nc.scalar.activation(tqT_s[:, ht, :], ps_tq, AF.Tanh,
                                     bias=cb_s[:, ht:ht + 1], scale=1.0)

        def q2():
            ps_sq = rowps.tile([BL, LQ], F32, tag="row", name="ps_sq")
            for b in range(BL):
                for ht in range(NHT):
                    nc.tensor.matmul(ps_sq, lhsT=vt1m_s[:, ht, b, :],
                                     rhs=tqT_s[:, ht, b * LQ:(b + 1) * LQ],
                                     start=(b == 0 and ht == 0),
                                     stop=(b == BL - 1 and ht == NHT - 1))
            nc.scalar.activation(esq, ps_sq, AF.Exp, accum_out=zq)
            nc.vector.reciprocal(rzq, zq)
            nc.vector.tensor_scalar(a_s, esq, rzq, None, op0=OP.mult)

        def q3():
            ps_at = trp.tile([LQ, BL], F16, tag="tr", name="ps_at")
            nc.tensor.transpose(ps_at, a_s, idh_s[:BL, :BL])
            nc.vector.tensor_mul(atm_s,
                                 bcast_dim(ps_at[:, :], 1, BL),
                                 colm_s[0:LQ, :, :])
            ps_rq = rowps.tile([BL, D], F32, tag="row", name="ps_rq")
            for b in range(BL):
                nc.tensor.matmul(ps_rq, lhsT=atm_s[:, b, :],
                                 rhs=qeN_s[:, b * D:(b + 1) * D],
                                 start=b == 0, stop=b == BL - 1)
            nc.vector.tensor_copy(rq1_s, ps_rq)

        def q4():
            for kt in range(NKT):
                ps_t = trp.tile([128, BL], F32, tag="tr", name="ps_q4")
                nc.tensor.transpose(ps_t, rq1_s[:, kt * 128:(kt + 1) * 128],
                                    idf_s[:BL, :BL])
                nc.vector.tensor_copy(rq1T_s[kt], ps_t)
                nc.vector.tensor_copy(rq1T8_s[kt // 2][:, kt % 2, 0:BL], ps_t)
            for ht in range(NHT):
                ps_w = trp.tile([128, BL], F32, tag="tr", name="ps_w0")
                for kt in range(NKT):
                    nc.tensor.matmul(ps_w,
                                     lhsT=WahT_s[:, kt, ht * 128:(ht + 1) * 128],
                                     rhs=rq1T_s[kt], start=kt == 0,
                                     stop=kt == NKT - 1)
                nc.vector.tensor_scalar(biasP_s[:, 0, ht, :], ps_w,
                                        wb_s[:, ht, :], None, op0=OP.add)

        def emit_gi():
            # giT[gate, b] = (rq1 @ wih.T).T / 16 + bih, fp8 DoubleRow,
            # all 8 batches at once in transposed layout
            for gt in range(12):
                ps_gi = rowps.tile([128, BL], F32, tag="row", name="ps_gi")
                for pr in range(NKT // 2):
                    nc.tensor.matmul(
                        ps_gi, lhsT=wihT_s[:, pr, :, gt * 128:(gt + 1) * 128],
                        rhs=rq1T8_s[pr][:, :, 0:BL],
                        start=pr == 0, stop=pr == NKT // 2 - 1, perf_mode=DR)
                nc.vector.tensor_scalar(giT_s[:, gt, :], ps_gi, 1.0 / 16.0,
                                        None, op0=OP.mult)
            nc.vector.tensor_add(giT_s, giT_s, bihT_s)

        # ================= emission =================
        q1()
        fetch_peC(2)
        fetch_pn(0, 0)
        em_a(0, 0)
        q2()
        fetch_peC(3)
        fetch_pn(0, 1)
        em_a(0, 1)
        q3()
        fetch_peC(4)
        fetch_pn(0, 2)
        em_a(0, 2)
        q4()
        em_b_tanh(0, 0)
        fetch_peC(5)
        fetch_pn(0, 3)
        em_a(0, 3)
        em_sp1(0, 0)
        em_exp1(0, 0)
        em_b_tanh(0, 1)
        em_sp1(0, 1)
        em_exp1(0, 1)
        em_b_tanh(0, 2)
        em_sp1(0, 2)
        em_exp1(0, 2)
        em_b_tanh(0, 3)
        em_sp1(0, 3)
        em_exp1(0, 3)
        emit_gi()
        group_end(0)

        for g in range(1, NG):
            # P2(g-1, 0) mini-iteration; em_a/tanh1/sp1 for (g, 0..1) were
            # emitted inside group_end(g-1), interleaved with the GRU
            fetch_pn(g, 1)
            em_p2_tanh(g - 1, 0)
            em_p2_mm(g - 1, 0)
            em_p2_exp(g - 1, 0)
            for pc in (2, 3):
                c = NPC * g + pc
                if c + 2 < NG * NPC:
                    fetch_peC(c + 2)
                fetch_pn(g, pc)
                em_sp1(g, pc - 1)
                em_exp1(g, pc - 1)
                em_a(g, pc)
                em_p2_tanh(g - 1, pc - 1)
                em_p2_mm(g - 1, pc - 1)
                em_b_tanh(g, pc)
                em_p2_exp(g - 1, pc - 1)
            em_sp1(g, NPC - 1)
            em_exp1(g, NPC - 1)
            em_p2_tanh(g - 1, NPC - 1)
            em_p2_mm(g - 1, NPC - 1)
            em_p2_exp(g - 1, NPC - 1)
            finish_p2(g - 1)
            group_end(g)

        # tail: last group's pointer step 2
        for pc in range(NPC):
            em_p2_tanh(NG - 1, pc)
            em_p2_mm(NG - 1, pc)
            em_p2_exp(NG - 1, pc)
        finish_p2(NG - 1)

        zlp.release()
        w2p.release()
        w1p.release()
        ctps.release()
        trp.release()
        rowps.release()
        ppp.release()
        grup.release()
        wmp.release()
        apb.release()
        t2bp.release()
        t2p.release()
        pnp.release()
        chunkp.release()
        sing.release()

    nc.compile()
    return nc


def _get_nc():
    global _CACHED_NC
    if _CACHED_NC is None:
        _CACHED_NC = _build()
    return _CACHED_NC


def _tiles(mat, nkt):  # [nkt*128, X] -> [128, nkt*X]
    x = mat.shape[1]
    return np.ascontiguousarray(
        mat.reshape(nkt, 128, x).transpose(1, 0, 2).reshape(128, nkt * x))


def _packA(f, Vt1, Vt2):
    # Vt1, Vt2: [BL, H] for this core's batch slice
    wp = np.zeros((128, WATOT), dtype=np.float16)

    def put(name, arr):
        o, ln = WA[name]
        assert arr.shape[1] == ln, (name, arr.shape, ln)
        wp[:arr.shape[0], o:o + ln] = arr

    put("WQvT", _tiles(f["WQv_W"].T.astype(np.float16), NHT))
    put("WQuT", _tiles(f["WQu_W"].T.astype(np.float16), NKT))
    put("WPhT", _tiles(f["WPh_W"].T.astype(np.float16), NKT))
    put("WahT", _tiles(f["Wah_W"].T.astype(np.float16), NKT))
    put("VQrT", _tiles(f["VQr"].reshape(1, H).T.astype(np.float16), NHT))
    # vt1m [128, ht, b, col]: col b = Vt1[b] per ht tile, rest zero
    v1 = np.zeros((128, NHT, BL, BL), dtype=np.float16)
    for b in range(BL):
        v1[:, :, b, b] = Vt1[b].reshape(NHT, 128).T
    put("vt1m", v1.reshape(128, NHT * BL * BL))
    # vt2g [128, ht, b, col]: col (b % GB) = Vt2[b], rest zero
    v2 = np.zeros((128, NHT, BL, GB), dtype=np.float16)
    for b in range(BL):
        v2[:, :, b, b % GB] = Vt2[b].reshape(NHT, 128).T
    put("vt2g", v2.reshape(128, NHT * BL * GB))
    put("idh", np.eye(128, dtype=np.float16))
    put("colm", np.broadcast_to(np.eye(BL, dtype=np.float16).reshape(1, BL * BL),
                                (128, BL * BL)))
    cm16 = np.hstack([np.eye(BL, dtype=np.float16),
                      np.zeros((BL, 16 - BL), dtype=np.float16)])
    put("cm16", np.broadcast_to(cm16.reshape(1, BL * 16), (128, BL * 16)))
    return wp


def _packG(f):
    # transposed gate biases, broadcast along the b (free) axis:
    # bhhT [128, 12, GB] then bihT [128, 12, BL]
    wp = np.zeros((128, 12 * (GB + BL)), dtype=np.float16)
    bhh = f["gru_bhh"].astype(np.float16).reshape(12, 128).T
    bih = f["gru_bih"].astype(np.float16).reshape(12, 128).T
    wp[:, 0:12 * GB] = np.repeat(bhh[:, :, None], GB, axis=2).reshape(128, -1)
    wp[:, 12 * GB:] = np.repeat(bih[:, :, None], BL, axis=2).reshape(128, -1)
    return wp


def _packQ(qe):
    wp = np.zeros((128, WQTOT), dtype=np.float16)
    o, ln = WQ["qeT"]
    qeT = np.ascontiguousarray(qe.transpose(2, 1, 0)).astype(np.float16)
    wp[:, o:o + ln] = _tiles(qeT.reshape(D, BL * LQ), NKT)
    return wp


def _packB(f):
    # x16 lifts the ~N(0, 0.05^2) weights out of fp8's subnormal range;
    # compensated on-chip (gi: x1/16 in the bias add; gh: cancels the
    # ct fp8 copy's 1/16 pre-scale)
    wp = np.zeros((128, WBTOT), dtype=np.float32)
    o, ln = WB["wihT"]
    wp[:, o:o + ln] = _tiles(f["gru_wih"].T.astype(np.float32) * 16.0, NKT)
    o, ln = WB["whhT"]
    wp[:, o:o + ln] = _tiles(f["gru_whh"].T.astype(np.float32) * 16.0, NKT)
    return _fp8(wp)


def _pack32(f):
    wp = np.zeros((128, W32TOT), dtype=np.float32)
    o, ln = W32["idf"]
    wp[:, o:o + ln] = np.eye(128, dtype=np.float32)
    o, ln = W32["cqb"]
    wp[:, o:o + ln] = (f["WQu_b"] + f["WQv_b"]).astype(np.float32).reshape(NHT, 128).T
    o, ln = W32["wb"]
    wp[:, o:o + ln] = (f["WPh_b"] + f["Wah_b"]).astype(np.float32).reshape(NHT, 128).T
    return wp


def _fp8(x):
    import ml_dtypes
    return np.ascontiguousarray(x).astype(ml_dtypes.float8_e4m3).view(np.uint8)


def make_in_maps(f):
    passEnc, quesEnc = f["passEnc"], f["quesEnc"]
    wp32 = _pack32(f)
    wpB = _packB(f)
    in_maps = []
    for i in range(NC):
        s = slice(i * BL, (i + 1) * BL)
        pe = passEnc[:, s, :]
        qe = quesEnc[:, s, :]
        wpA_ = _packA(f, f["Vt1"][s, :, 0], f["Vt2"][s, :, 0])
        wpQ_ = _packQ(qe)
        # peC [g, pc, part, b', kt, d]: per-partition runs of 8KB
        peC = np.ascontiguousarray(
            pe.astype(np.float16).reshape(NPC, 512, NG, GB, NKT, 128).transpose(
                2, 0, 5, 3, 4, 1))
        # peN8 [g, j, part, i, b', sub, d]: global p = (2j+i)*256 + sub*128
        # + part; per-partition contiguous runs of 4KB
        peN8 = _fp8(pe.reshape(NPR // 2, 2, 2, 128, NG, GB, D).transpose(
            4, 0, 3, 1, 5, 2, 6))
        in_maps.append({
            "peC": peC,
            "peN8": peN8,
            "wpA": wpA_, "wpQ": wpQ_, "wpB": wpB, "wp32": wp32,
            "wpN": qe.astype(np.float16).reshape(LQ, BL * D),
            "wpG": _packG(f),
        })
    return in_maps


def kernel(**inputs):
    f = {k: np.asarray(v) for k, v in inputs.items()}
    in_maps = make_in_maps(f)
    nc = _get_nc()
    res = run_bass_kernel_spmd(nc, in_maps, core_ids=list(range(NC)))
    aP1 = np.concatenate([res.results[i]["out"][0] for i in range(NC)], axis=0)
    aP2 = np.concatenate([res.results[i]["out"][1] for i in range(NC)], axis=0)
    return (aP1.astype(np.float32), aP2.astype(np.float32))


# revision 42
# speedup vs baseline: 1.1081x; 1.0253x over previous
"""Answer-pointer network forward pass on 8 TRN2 NeuronCores.

Data-parallel over batch: B=64 -> 8 batches per core, weights replicated.
No collectives; each core computes softmax attention maps (aP1, aP2) for
its batch shard and the host concatenates.

Schedule: the 8 per-core batches are split into G=4 groups of GB=2 and
software-pipelined. Nothing couples batches mathematically (softmax, ct,
GRU are all per-batch), so group g's pointer-step-2 (scalar-engine-bound:
tanh over [LP, GB, H]) runs concurrently with group g+1's pointer-step-1
(tensor-engine-bound: the WPh linear over [LP, GB, 2H]). This hides the
~37us of step-2 tanh that a batch-monolithic schedule serializes after
the GRU, leaving only the last group's step-2 exposed at the tail.

Within a group, P1 is chunk-pipelined over 4 p-chunks of 512: passP
matmuls (tensor) -> PSUM->SBUF copies (vector/gpsimd) -> tanh (scalar)
-> masked sP matmuls (tensor) -> per-chunk exp with Z-accumulation
(scalar). sP/exp for chunk pc are emitted one iteration later than the
chunk's tanh so no engine queue ever heads into a not-yet-satisfied
dependency (engines execute their queues in order; a stalled head op
blocks ready work behind it).

Layouts (host-side prep, outside HW exec):
  - peC  [pc, b, 128, kt, 512] fp16: passEnc feature-major for the WPh
    linear (contract over d=512 on partitions).
  - peN8 [pr, 128, b, 2, 512] fp8e4: passEnc position-major for the
    attention-weighted context ct, pre-packed for fp8 DoubleRow matmuls
    (contract over p=2048; each DR instruction consumes K=256).

Measured PE cost law: a matmul instruction costs ~N_out_columns x
0.417ns at full p-state; the tensor engine clock ramps with sustained
use (idle gaps drop it to ~0.83ns/col for up to 3us), so the schedule
aims to keep the PE continuously fed.

The GRU consumes the *unnormalized* context sum (matmul is linear) so
its matmuls start before the softmax normalizer 1/Z is ready; 1/Z is
folded into the gate bias-add. Sigmoid is computed as 0.5*tanh(x/2)+0.5
so every activation in the kernel (tanh/exp/identity) lives in the one
'exp_and_others' table -- zero ACT_TABLE_LOAD switches after the first.

Per-batch reductions (sP, sQ, rQ, ct) use masked stationary operands:
column (b mod GB) of the lhsT kept, rest zeroed, so batch b's matmul
writes only PSUM row (b mod GB); accumulating over the group assembles
[GB, N] without partition-offset copies.
"""

import numpy as np

try:
    import concourse.bass as bass
except ImportError:  # pragma: no cover
    import sys

    sys.path.insert(0, "/opt/trn_rl_repo")
    import concourse.bass as bass

import concourse.tile as tile
from concourse import bacc, mybir
from concourse.bass_utils import run_bass_kernel_spmd

F8 = mybir.dt.float8e4
F16 = mybir.dt.float16
F32 = mybir.dt.float32
AF = mybir.ActivationFunctionType
OP = mybir.AluOpType
DR = mybir.MatmulPerfMode.DoubleRow

H = 256      # hidden
D = 512      # 2*hidden
LP = 2048    # passage length
LQ = 64      # question length
B = 64       # global batch
BL = 8       # batch per core
G6 = 6 * H   # 1536, GRU gate width
NC = 8       # cores
NKT = D // 128    # 4 contraction tiles over d
NHT = H // 128    # 2 tiles over h
NPC = LP // 512   # 4 p-chunks of 512
NPR = LP // 256   # 8 p-pairs of 256 (DoubleRow K tiles)
NG = 4            # batch groups per core
GB = BL // NG     # 2 batches per group


def _layout(entries):
    off, table = 0, {}
    for name, ln in entries:
        table[name] = (off, ln)
        off += ln
    return table, off


# hot entries (Q-phase) first: they arrive in a separate, earlier DMA
WA, WATOT = _layout([
    ("WQvT", NHT * H), ("WQuT", NKT * H), ("VQrT", NHT),
    ("vt1m", NHT * BL * BL),
    ("idh", 128), ("colm", BL * BL), ("cm16", BL * 16),
    ("WPhT", NKT * H), ("WahT", NKT * H), ("vt2g", NHT * BL * GB),
])
WAHOT = WA["idh"][0]   # first four entries form the hot prefix
WQ, WQTOT = _layout([("qeT", NKT * BL * LQ)])
# GRU weight matrices in fp8 (inputs rq1/ct are small weighted averages;
# quantization noise lands ~5e-4 on the final softmax)
WB, WBTOT = _layout([("wihT", NKT * G6), ("whhT", NKT * G6)])
W32, W32TOT = _layout([("idf", 128), ("cqb", NHT), ("wb", NHT)])

_CACHED_NC = None


def _build():
    nc = bacc.Bacc("TRN2", target_bir_lowering=False, debug=False, num_devices=NC)

    peC = nc.dram_tensor("peC", [NG, NPC, 128, GB, NKT, 512], F16,
                         kind="ExternalInput").ap()
    peN8 = nc.dram_tensor("peN8", [NG, NPR // 2, 128, 2, GB, 2, 512], F8,
                         kind="ExternalInput").ap()
    wpA = nc.dram_tensor("wpA", [128, WAHOT], F16, kind="ExternalInput").ap()
    wpA2 = nc.dram_tensor("wpA2", [128, WATOT - WAHOT], F16,
                          kind="ExternalInput").ap()
    wpQ = nc.dram_tensor("wpQ", [128, WQTOT], F16, kind="ExternalInput").ap()
    wpN = nc.dram_tensor("wpN", [LQ, BL * D], F16, kind="ExternalInput").ap()
    wpG = nc.dram_tensor("wpG", [128, 12 * (GB + BL)], F16,
                         kind="ExternalInput").ap()
    wpB = nc.dram_tensor("wpB", [128, WBTOT], F8, kind="ExternalInput").ap()
    wp32 = nc.dram_tensor("wp32", [128, W32TOT], F32, kind="ExternalInput").ap()
    out = nc.dram_tensor("out", [2, BL, LP], F32, kind="ExternalOutput").ap()

    with tile.TileContext(nc) as tc:
        sing = tc.alloc_tile_pool(name="sing", bufs=1)

        def _single(shape, dtype, name):
            return sing.tile(shape, dtype, name=name, tag=name)

        chunkp = tc.alloc_tile_pool(name="chunk", bufs=3)
        pnp = tc.alloc_tile_pool(name="pn", bufs=4)
        t2p = tc.alloc_tile_pool(name="t2", bufs=3)
        t2bp = tc.alloc_tile_pool(name="t2b", bufs=3)
        apb = tc.alloc_tile_pool(name="apb", bufs=2)
        wmp = tc.alloc_tile_pool(name="wm", bufs=2)
        grup = tc.alloc_tile_pool(name="gru", bufs=1)
        # PSUM budget: ppp 3 banks + rowps 3 + trp 1 + ctps 1 = 8
        ppp = tc.alloc_tile_pool(name="ppp", bufs=3, space="PSUM")
        rowps = tc.alloc_tile_pool(name="rowps", bufs=3, space="PSUM")
        trp = tc.alloc_tile_pool(name="trp", bufs=1, space="PSUM")
        ctps = tc.alloc_tile_pool(name="ctps", bufs=1, space="PSUM")

        # ---- packed weights, hot-first ----
        wpA_s = _single([128, WATOT], F16, "wpA_s")
        nc.sync.dma_start(wpA_s[:, 0:WAHOT], wpA)
        pe_tiles = {}
        pn_tiles = {}

        def fetch_peC(c):
            g, pc = divmod(c, NPC)
            t = chunkp.tile([128, GB, NKT, 512], F16, tag="pe", name=f"pe{c}")
            eng = nc.sync if c % 2 == 0 else nc.gpsimd
            eng.dma_start(t, peC[g, pc])
            pe_tiles[(g, pc)] = t

        def fetch_pn(g, j):
            t = pnp.tile([128, 2, GB, 2, 512], F8, tag="pn", name=f"pn{g}_{j}")
            nc.sync.dma_start(t, peN8[g, j])
            pn_tiles[(g, j)] = t

        wpQ_s = _single([128, WQTOT], F16, "wpQ_s")
        nc.sync.dma_start(wpQ_s, wpQ)
        wp32_s = _single([128, W32TOT], F32, "wp32_s")
        nc.sync.dma_start(wp32_s, wp32)
        nc.sync.dma_start(wpA_s[:, WAHOT:WATOT], wpA2)
        fetch_peC(0)
        wpN_s = _single([LQ, BL * D], F16, "wpN_s")
        nc.gpsimd.dma_start(wpN_s, wpN)
        fetch_peC(1)
        wpB_s = _single([128, WBTOT], F8, "wpB_s")
        nc.gpsimd.dma_start(wpB_s, wpB)
        wpG_s = _single([128, 12 * (GB + BL)], F16, "wpG_s")
        nc.gpsimd.dma_start(wpG_s, wpG)

        def sA(name):
            o, ln = WA[name]
            return wpA_s[:, o:o + ln]

        WQvT_s = sA("WQvT").rearrange("p (kt h) -> p kt h", kt=NHT)
        WQuT_s = sA("WQuT").rearrange("p (kt h) -> p kt h", kt=NKT)
        WPhT_s = sA("WPhT").rearrange("p (kt h) -> p kt h", kt=NKT)
        WahT_s = sA("WahT").rearrange("p (kt h) -> p kt h", kt=NKT)
        VQrT_s = sA("VQrT").rearrange("p (ht o) -> p ht o", ht=NHT)
        vt1m_s = sA("vt1m").rearrange("p (ht b c) -> p ht b c", ht=NHT, b=BL)
        vt2g_s = sA("vt2g").rearrange("p (ht b c) -> p ht b c", ht=NHT, b=BL)
        idh_s = sA("idh")
        colm_s = sA("colm").rearrange("p (b c) -> p b c", b=BL)
        cm16_s = sA("cm16").rearrange("p (b m) -> p b m", b=BL)
        qeT_s = wpQ_s[:, WQ["qeT"][0]:WQ["qeT"][0] + NKT * BL * LQ].rearrange(
            "p (kt bq) -> p kt bq", kt=NKT)
        qeN_s = wpN_s[:, :]
        wihT_s = wpB_s[:, WB["wihT"][0]:WB["wihT"][0] + NKT * G6].rearrange(
            "p (pr sub g) -> p pr sub g", pr=NKT // 2, sub=2)
        whhT_s = wpB_s[:, WB["whhT"][0]:WB["whhT"][0] + NKT * G6].rearrange(
            "p (pr sub g) -> p pr sub g", pr=NKT // 2, sub=2)
        bhhT_s = wpG_s[:, 0:12 * GB].rearrange("p (gt c) -> p gt c", gt=12)
        bihT_s = wpG_s[:, 12 * GB:12 * (GB + BL)].rearrange(
            "p (gt c) -> p gt c", gt=12)
        idf_s = wp32_s[:, W32["idf"][0]:W32["idf"][0] + 128]
        cqb_s = wp32_s[:, W32["cqb"][0]:W32["cqb"][0] + NHT]
        wb_s = wp32_s[:, W32["wb"][0]:W32["wb"][0] + NHT].rearrange(
            "p (ht o) -> p ht o", ht=NHT)

        # persistent activations
        ppr_s = _single([128, NHT, BL, NPC, 512], F16, "ppr_s")  # raw passP
        biasP_s = _single([128, 2, NHT, BL], F32, "biasP_s")
        # per-group exp(sP) rows at base partition 0 (matmul/transpose
        # inputs must start at partition 0/32/64)
        w1p = tc.alloc_tile_pool(name="w1p", bufs=2)
        w2p = tc.alloc_tile_pool(name="w2p", bufs=2)
        zlp = tc.alloc_tile_pool(name="zlp", bufs=2)
        w1_t, w2_t = {}, {}
        # engine accesses must start at partition 0 (mult-of-32), so all
        # per-group [GB, ...] data lives in its own base-0 tile
        zpart_t, zp2_t, z1_t, rz1_t, z2_t, rz2_t = ({} for _ in range(6))
        rq1_s = _single([BL, D], F32, "rq1_s")
        rq1T_s = [_single([128, BL], F16, f"rq1T{k}") for k in range(NKT)]
        rq1T8_s = [_single([128, 2, 16], F8, f"rq1T8{k}") for k in range(NKT // 2)]
        giT_s = _single([128, 12, BL], F16, "giT_s")
        colm8_s = _single([128, BL, 16], F8, "colm8_s")
        nc.vector.tensor_copy(colm8_s, cm16_s)

        def bcast_dim(ap, axis, size):
            """Insert a stride-0 (broadcast) free dim at position axis."""
            entries = list(ap.ap)
            entries.insert(axis, [0, size])
            return bass.AP(tensor=ap.tensor, offset=ap.offset, ap=entries)

        # ================= pipelined P1 / P2 stages =================
        t1saved, t2saved, sp1_ps, sp2_ps = {}, {}, {}, {}

        def em_a(g, pc, scalar_copies=False):
            """passP matmuls for group g, chunk pc; copies PSUM -> ppr.
            At group boundaries the copies go on the scalar engine (idle
            while waiting for the GRU gate chain) so the vector queue
            stays clear for that chain."""
            pe = pe_tiles.pop((g, pc))
            for b in range(GB):
                pps = [ppp.tile([128, 512], F32, tag="acc", name=f"pp{ht}")
                       for ht in range(NHT)]
                for kt in range(NKT):
                    for ht in range(NHT):
                        nc.tensor.matmul(pps[ht],
                                         lhsT=WPhT_s[:, kt, ht * 128:(ht + 1) * 128],
                                         rhs=pe[:, b, kt, :],
                                         start=kt == 0, stop=kt == NKT - 1)
                for ht in range(NHT):
                    dst = ppr_s[:, ht, GB * g + b, pc, :]
                    if scalar_copies:
                        nc.scalar.activation(dst, pps[ht], AF.Copy)
                    else:
                        nc.vector.tensor_copy(dst, pps[ht])

        def em_b_tanh(g, pc):
            for b in range(GB):
                t2 = t2p.tile([128, NHT, 512], F16, tag="t2", name="t2")
                for ht in range(NHT):
                    nc.scalar.activation(t2[:, ht, :],
                                         ppr_s[:, ht, GB * g + b, pc, :], AF.Tanh,
                                         bias=biasP_s[:, 0, ht,
                                                      GB * g + b:GB * g + b + 1],
                                         scale=1.0)
                t1saved[(g, pc, b)] = t2

        def em_sp1(g, pc):
            ps = rowps.tile([GB, 512], F32, tag="row", name=f"sp1_{pc}")
            for b in range(GB):
                t2 = t1saved.pop((g, pc, b))
                for ht in range(NHT):
                    nc.tensor.matmul(ps, lhsT=vt2g_s[:, ht, GB * g + b, :],
                                     rhs=t2[:, ht, :],
                                     start=(b == 0 and ht == 0),
                                     stop=(b == GB - 1 and ht == NHT - 1))
            sp1_ps[(g, pc)] = ps

        def em_exp1(g, pc):
            ps = sp1_ps.pop((g, pc))
            if pc == 0:
                w1_t[g] = w1p.tile([GB, LP], F16, tag="w1", name=f"w1_{g}")
                zpart_t[g] = zlp.tile([GB, NPC], F32, tag="zp1", name=f"zp1_{g}")
            nc.scalar.activation(w1_t[g][:, pc * 512:(pc + 1) * 512], ps, AF.Exp,
                                 accum_out=zpart_t[g][:, pc:pc + 1])

        def em_p2_tanh(g, pc):
            for b in range(GB):
                t2 = t2bp.tile([128, NHT, 512], F16, tag="t2b", name="t2b")
                for ht in range(NHT):
                    nc.scalar.activation(t2[:, ht, :],
                                         ppr_s[:, ht, GB * g + b, pc, :], AF.Tanh,
                                         bias=biasP_s[:, 1, ht,
                                                      GB * g + b:GB * g + b + 1],
                                         scale=1.0)
                t2saved[(g, pc, b)] = t2

        def em_p2_mm(g, pc):
            ps = rowps.tile([GB, 512], F32, tag="row", name=f"sp2_{pc}")
            for b in range(GB):
                t2 = t2saved.pop((g, pc, b))
                for ht in range(NHT):
                    nc.tensor.matmul(ps, lhsT=vt2g_s[:, ht, GB * g + b, :],
                                     rhs=t2[:, ht, :],
                                     start=(b == 0 and ht == 0),
                                     stop=(b == GB - 1 and ht == NHT - 1))
            sp2_ps[(g, pc)] = ps

        def em_p2_exp(g, pc):
            ps = sp2_ps.pop((g, pc))
            if pc == 0:
                w2_t[g] = w2p.tile([GB, LP], F16, tag="w2", name=f"w2_{g}")
                zp2_t[g] = zlp.tile([GB, NPC], F32, tag="zp2", name=f"zp2_{g}")
            nc.scalar.activation(w2_t[g][:, pc * 512:(pc + 1) * 512], ps, AF.Exp,
                                 accum_out=zp2_t[g][:, pc:pc + 1])

        def finish_p2(g):
            rows = slice(GB * g, GB * (g + 1))
            z2 = zlp.tile([GB, 1], F32, tag="z2", name=f"z2_{g}")
            rz2 = zlp.tile([GB, 1], F32, tag="rz2", name=f"rz2_{g}")
            nc.vector.reduce_sum(z2, zp2_t.pop(g), axis=mybir.AxisListType.X)
            nc.vector.reciprocal(rz2, z2)
            w2g = w2_t.pop(g)
            for h in range(2):
                hs = slice(h * LP // 2, (h + 1) * LP // 2)
                apc = apb.tile([GB, LP // 2], F32, tag="ap", name="ap2c")
                nc.vector.tensor_scalar(apc, w2g[:, hs], rz2, None,
                                        op0=OP.mult)
                nc.sync.dma_start(out=out[1, rows, hs], in_=apc)

        # ================= group epilogue: softmax Z, ct, GRU =============
        grust = {}

        def gru_a(g, ps_ct):
            """ct normalize + transpose + ghT matmuls. 1/Z is folded into
            the ct PSUM->SBUF copy (per-partition scale in [GB, D] layout),
            so everything downstream uses normalized ct."""
            ctn = grup.tile([GB, D], F16, tag="ctn", name=f"ctn{g}")
            nc.vector.tensor_scalar(ctn, ps_ct, rz1_t[g], None, op0=OP.mult)
            ctT16 = grup.tile([128, NKT, GB], F16, tag="ctT16", name=f"ctT16{g}")
            ctT8 = [grup.tile([128, 2, 16], F8, tag=f"ctT8{j}", name=f"ctT8_{g}{j}")
                    for j in range(NKT // 2)]
            for kt in range(NKT):
                ps_t = trp.tile([128, GB], F16, tag="tr", name="ps_t")
                nc.tensor.transpose(ps_t, ctn[:, kt * 128:(kt + 1) * 128],
                                    idh_s[:GB, :GB])
                nc.vector.tensor_copy(ctT16[:, kt, :], ps_t)
                nc.vector.tensor_copy(ctT8[kt // 2][:, kt % 2, 0:GB], ps_t)
            # ghT[gate, b] = (ct_norm @ whh.T).T / 16 + bhh  (weights are x16)
            ghT = grup.tile([128, 12, GB], F16, tag="ghT", name=f"ghT{g}")
            for gt in range(12):
                ps_g = rowps.tile([128, GB], F32, tag="row", name="ps_g")
                for p2 in range(NKT // 2):
                    nc.tensor.matmul(
                        ps_g, lhsT=whhT_s[:, p2, :, gt * 128:(gt + 1) * 128],
                        rhs=ctT8[p2][:, :, 0:GB],
                        start=p2 == 0, stop=p2 == NKT // 2 - 1, perf_mode=DR)
                nc.vector.tensor_scalar(ghT[:, gt, :], ps_g, 1.0 / 16.0, None,
                                        op0=OP.mult)
            nc.vector.tensor_add(ghT, ghT, bhhT_s)
            grust[g] = (ctT16, ghT)

        def gru_b(g):
            """gate elementwise chain in transposed layout ([128, <=12, GB]:
            free size <= 24 per op). r,z = sigmoid(gi+gh) computed as
            0.5*tanh(x/2)+0.5 (stays in the exp/tanh activation table ->
            no ACT_TABLE_LOAD switches). Gate rows: 0:4 = r, 4:8 = z,
            8:12 = n."""
            ctT16, ghT = grust[g]
            giTg = giT_s[:, :, slice(GB * g, GB * (g + 1))]
            rzin = grup.tile([128, 8, GB], F16, tag="rzin", name="rzin")
            nc.vector.tensor_add(rzin, giTg[:, 0:8, :], ghT[:, 0:8, :])
            th = grup.tile([128, 8, GB], F16, tag="th", name="th")
            nc.scalar.activation(th, rzin, AF.Tanh, scale=0.5)
            # n = tanh(gi_n + r*gh_n); r*gh_n = 0.5*(th_r*gh_n + gh_n)
            a_t = grup.tile([128, 4, GB], F16, tag="ga", name="ga")
            nc.vector.tensor_mul(a_t, th[:, 0:4, :], ghT[:, 8:12, :])
            nc.vector.tensor_add(a_t, a_t, ghT[:, 8:12, :])
            nin = grup.tile([128, 4, GB], F16, tag="nin", name="nin")
            nc.vector.scalar_tensor_tensor(nin, a_t, 0.5, giTg[:, 8:12, :],
                                           op0=OP.mult, op1=OP.add)
            n_t = grup.tile([128, 4, GB], F16, tag="gn", name="gn")
            nc.scalar.activation(n_t, nin, AF.Tanh)
            # h' = n + z*(ct-n); z*(ct-n) = 0.5*(th_z*d + d), d = ctT - n
            d_t = grup.tile([128, 4, GB], F16, tag="gd", name="gd")
            nc.vector.tensor_sub(d_t, ctT16, n_t)
            e_t = grup.tile([128, 4, GB], F16, tag="ge", name="ge")
            nc.vector.tensor_mul(e_t, th[:, 4:8, :], d_t)
            nc.vector.tensor_add(e_t, e_t, d_t)
            rq2T = grup.tile([128, 4, GB], F16, tag="rq2T", name="rq2T")
            nc.vector.scalar_tensor_tensor(rq2T, e_t, 0.5, n_t,
                                           op0=OP.mult, op1=OP.add)
            grust[g] = rq2T

        def gru_c(g):
            rq2T = grust.pop(g)
            rows = slice(GB * g, GB * (g + 1))
            for ht in range(NHT):
                ps_w = trp.tile([128, GB], F32, tag="tr", name="ps_w")
                for kt in range(NKT):
                    nc.tensor.matmul(ps_w,
                                     lhsT=WahT_s[:, kt, ht * 128:(ht + 1) * 128],
                                     rhs=rq2T[:, kt, :], start=kt == 0,
                                     stop=kt == NKT - 1)
                nc.vector.tensor_scalar(biasP_s[:, 1, ht, rows], ps_w,
                                        wb_s[:, ht, :], None, op0=OP.add)

        def group_end(g):
            rows = slice(GB * g, GB * (g + 1))
            if g + 1 < NG:
                fetch_peC(NPC * (g + 1) + 2)
            # 1/Z for step 1: ready while the ct matmuls run
            z1_t[g] = zlp.tile([GB, 1], F32, tag="z1", name=f"z1_{g}")
            rz1_t[g] = zlp.tile([GB, 1], F32, tag="rz1", name=f"rz1_{g}")
            nc.vector.reduce_sum(z1_t[g], zpart_t.pop(g),
                                 axis=mybir.AxisListType.X)
            nc.vector.reciprocal(rz1_t[g], z1_t[g])
            # ct += w1[rows] @ passEnc, via fp8 DoubleRow
            ps_ct = ctps.tile([GB, D], F32, tag="ct", name=f"ct{g}")
            for j in range(NPR // 2):
                pn = pn_tiles.pop((g, j))
                for i in range(2):
                    pr = 2 * j + i
                    wm8 = wmp.tile([128, 2, GB, 16], F8, tag="wm", name="wm8")
                    for sub in range(2):
                        pt = pr * 2 + sub
                        ps_wt = trp.tile([128, GB], F16, tag="tr", name="ps_wt")
                        nc.tensor.transpose(
                            ps_wt, w1_t[g][:, pt * 128:(pt + 1) * 128],
                            idh_s[:GB, :GB])
                        nc.vector.tensor_mul(wm8[:, sub],
                                             bcast_dim(ps_wt[:, :], 2, 16),
                                             colm8_s[:, 0:GB, :])
                    for b in range(GB):
                        nc.tensor.matmul(
                            ps_ct, lhsT=wm8[:, :, b, 0:GB], rhs=pn[:, i, b, :, :],
                            start=(pr == 0 and b == 0),
                            stop=(pr == NPR - 1 and b == GB - 1),
                            perf_mode=DR)
            # the GRU serial chain is interleaved with the next group's
            # first two passP blocks so the tensor queue never runs dry;
            # tensor order: stage_c -> em_a(0) -> ct transposes + ghT ->
            # em_a(1) (covers the gate elementwise chain) -> wah -> sp1
            if g + 1 < NG:
                fetch_pn(g + 1, 0)
                em_a(g + 1, 0, scalar_copies=True)
            gru_a(g, ps_ct)
            gru_b(g)
            if g + 1 < NG:
                fetch_peC(NPC * (g + 1) + 3)
                em_a(g + 1, 1)
            gru_c(g)
            if g + 1 < NG:
                em_b_tanh(g + 1, 0)
                em_sp1(g + 1, 0)
                em_exp1(g + 1, 0)
                em_b_tanh(g + 1, 1)
            # aP1 normalize + store: off the critical path
            w1g = w1_t.pop(g)
            for h in range(2):
                hs = slice(h * LP // 2, (h + 1) * LP // 2)
                apc = apb.tile([GB, LP // 2], F32, tag="ap", name="ap1c")
                nc.vector.tensor_scalar(apc, w1g[:, hs], rz1_t[g], None,
                                        op0=OP.mult)
                nc.gpsimd.dma_start(out=out[0, rows, hs], in_=apc)

        # ---- Q phase        # ---- Q phase: question-aware initial state rQ, all 8 batches ----
        tqT_s = _single([128, NHT, BL * LQ], F16, "tqT_s")
        cb_s = _single([128, NHT], F32, "cb_s")
        esq = _single([BL, LQ], F32, "esq")
        zq = _single([BL, 1], F32, "zq")
        rzq = _single([BL, 1], F32, "rzq")
        a_s = _single([BL, LQ], F16, "a_s")
        atm_s = _single([LQ, BL, BL], F16, "atm_s")

        def q1():
            ps_qv = trp.tile([128, NHT], F32, tag="tr", name="ps_qv")
            for ht in range(NHT):
                for kt in range(NHT):
                    nc.tensor.matmul(ps_qv[:, ht:ht + 1],
                                     lhsT=WQvT_s[:, kt, ht * 128:(ht + 1) * 128],
                                     rhs=VQrT_s[:, kt, :], start=kt == 0,
                                     stop=kt == NHT - 1)
            nc.vector.tensor_add(cb_s, ps_qv, cqb_s)
            for ht in range(NHT):
                ps_tq = ppp.tile([128, 512], F32, tag="acc", name="ps_tq")
                for kt in range(NKT):
                    nc.tensor.matmul(ps_tq,
                                     lhsT=WQuT_s[:, kt, ht * 128:(ht + 1) * 128],
                                     rhs=qeT_s[:, kt, :], start=kt == 0,
                                     stop=kt == NKT - 1)
                nc.scalar.activation(tqT_s[:, ht, :], ps_tq, AF.Tanh,
                                     bias=cb_s[:, ht:ht + 1], scale=1.0)

        def q2():
            ps_sq = rowps.tile([BL, LQ], F32, tag="row", name="ps_sq")
            for b in range(BL):
                for ht in range(NHT):
                    nc.tensor.matmul(ps_sq, lhsT=vt1m_s[:, ht, b, :],
                                     rhs=tqT_s[:, ht, b * LQ:(b + 1) * LQ],
                                     start=(b == 0 and ht == 0),
                                     stop=(b == BL - 1 and ht == NHT - 1))
            nc.scalar.activation(esq, ps_sq, AF.Exp, accum_out=zq)
            nc.vector.reciprocal(rzq, zq)
            nc.vector.tensor_scalar(a_s, esq, rzq, None, op0=OP.mult)

        def q3():
            ps_at = trp.tile([LQ, BL], F16, tag="tr", name="ps_at")
            nc.tensor.transpose(ps_at, a_s, idh_s[:BL, :BL])
            nc.vector.tensor_mul(atm_s,
                                 bcast_dim(ps_at[:, :], 1, BL),
                                 colm_s[0:LQ, :, :])
            ps_rq = rowps.tile([BL, D], F32, tag="row", name="ps_rq")
            for b in range(BL):
                nc.tensor.matmul(ps_rq, lhsT=atm_s[:, b, :],
                                 rhs=qeN_s[:, b * D:(b + 1) * D],
                                 start=b == 0, stop=b == BL - 1)
            nc.vector.tensor_copy(rq1_s, ps_rq)

        def q4():
            for kt in range(NKT):
                ps_t = trp.tile([128, BL], F32, tag="tr", name="ps_q4")
                nc.tensor.transpose(ps_t, rq1_s[:, kt * 128:(kt + 1) * 128],
                                    idf_s[:BL, :BL])
                nc.vector.tensor_copy(rq1T_s[kt], ps_t)
                nc.vector.tensor_copy(rq1T8_s[kt // 2][:, kt % 2, 0:BL], ps_t)
            for ht in range(NHT):
                ps_w = trp.tile([128, BL], F32, tag="tr", name="ps_w0")
                for kt in range(NKT):
                    nc.tensor.matmul(ps_w,
                                     lhsT=WahT_s[:, kt, ht * 128:(ht + 1) * 128],
                                     rhs=rq1T_s[kt], start=kt == 0,
                                     stop=kt == NKT - 1)
                nc.vector.tensor_scalar(biasP_s[:, 0, ht, :], ps_w,
                                        wb_s[:, ht, :], None, op0=OP.add)

        def emit_gi():
            # giT[gate, b] = (rq1 @ wih.T).T / 16 + bih, fp8 DoubleRow,
            # all 8 batches at once in transposed layout
            for gt in range(12):
                ps_gi = rowps.tile([128, BL], F32, tag="row", name="ps_gi")
                for pr in range(NKT // 2):
                    nc.tensor.matmul(
                        ps_gi, lhsT=wihT_s[:, pr, :, gt * 128:(gt + 1) * 128],
                        rhs=rq1T8_s[pr][:, :, 0:BL],
                        start=pr == 0, stop=pr == NKT // 2 - 1, perf_mode=DR)
                nc.vector.tensor_scalar(giT_s[:, gt, :], ps_gi, 1.0 / 16.0,
                                        None, op0=OP.mult)
            nc.vector.tensor_add(giT_s, giT_s, bihT_s)

        # ================= emission =================
        q1()
        fetch_peC(2)
        fetch_pn(0, 0)
        em_a(0, 0)
        q2()
        fetch_peC(3)
        fetch_pn(0, 1)
        em_a(0, 1)
        q3()
        fetch_peC(4)
        fetch_pn(0, 2)
        em_a(0, 2)
        q4()
        em_b_tanh(0, 0)
        fetch_peC(5)
        fetch_pn(0, 3)
        em_a(0, 3)
        em_sp1(0, 0)
        em_exp1(0, 0)
        em_b_tanh(0, 1)
        em_sp1(0, 1)
        em_exp1(0, 1)
        em_b_tanh(0, 2)
        em_sp1(0, 2)
        em_exp1(0, 2)
        em_b_tanh(0, 3)
        em_sp1(0, 3)
        em_exp1(0, 3)
        emit_gi()
        group_end(0)

        for g in range(1, NG):
            # P2(g-1, 0) mini-iteration; em_a/tanh1/sp1 for (g, 0..1) were
            # emitted inside group_end(g-1), interleaved with the GRU
            fetch_pn(g, 1)
            em_p2_tanh(g - 1, 0)
            em_p2_mm(g - 1, 0)
            em_p2_exp(g - 1, 0)
            for pc in (2, 3):
                c = NPC * g + pc
                if c + 2 < NG * NPC:
                    fetch_peC(c + 2)
                fetch_pn(g, pc)
                em_sp1(g, pc - 1)
                em_exp1(g, pc - 1)
                em_a(g, pc)
                em_p2_tanh(g - 1, pc - 1)
                em_p2_mm(g - 1, pc - 1)
                em_b_tanh(g, pc)
                em_p2_exp(g - 1, pc - 1)
            em_sp1(g, NPC - 1)
            em_exp1(g, NPC - 1)
            em_p2_tanh(g - 1, NPC - 1)
            em_p2_mm(g - 1, NPC - 1)
            em_p2_exp(g - 1, NPC - 1)
            finish_p2(g - 1)
            group_end(g)

        # tail: last group's pointer step 2
        for pc in range(NPC):
            em_p2_tanh(NG - 1, pc)
            em_p2_mm(NG - 1, pc)
            em_p2_exp(NG - 1, pc)
        finish_p2(NG - 1)

        zlp.release()
        w2p.release()
        w1p.release()
        ctps.release()
        trp.release()
        rowps.release()
        ppp.release()
        grup.release()
        wmp.release()
        apb.release()
        t2bp.release()
        t2p.release()
        pnp.release()
        chunkp.release()
        sing.release()

    nc.compile()
    return nc


def _get_nc():
    global _CACHED_NC
    if _CACHED_NC is None:
        _CACHED_NC = _build()
    return _CACHED_NC


def _tiles(mat, nkt):  # [nkt*128, X] -> [128, nkt*X]
    x = mat.shape[1]
    return np.ascontiguousarray(
        mat.reshape(nkt, 128, x).transpose(1, 0, 2).reshape(128, nkt * x))


def _packA(f, Vt1, Vt2):
    # Vt1, Vt2: [BL, H] for this core's batch slice
    wp = np.zeros((128, WATOT), dtype=np.float16)

    def put(name, arr):
        o, ln = WA[name]
        assert arr.shape[1] == ln, (name, arr.shape, ln)
        wp[:arr.shape[0], o:o + ln] = arr

    put("WQvT", _tiles(f["WQv_W"].T.astype(np.float16), NHT))
    put("WQuT", _tiles(f["WQu_W"].T.astype(np.float16), NKT))
    put("WPhT", _tiles(f["WPh_W"].T.astype(np.float16), NKT))
    put("WahT", _tiles(f["Wah_W"].T.astype(np.float16), NKT))
    put("VQrT", _tiles(f["VQr"].reshape(1, H).T.astype(np.float16), NHT))
    # vt1m [128, ht, b, col]: col b = Vt1[b] per ht tile, rest zero
    v1 = np.zeros((128, NHT, BL, BL), dtype=np.float16)
    for b in range(BL):
        v1[:, :, b, b] = Vt1[b].reshape(NHT, 128).T
    put("vt1m", v1.reshape(128, NHT * BL * BL))
    # vt2g [128, ht, b, col]: col (b % GB) = Vt2[b], rest zero
    v2 = np.zeros((128, NHT, BL, GB), dtype=np.float16)
    for b in range(BL):
        v2[:, :, b, b % GB] = Vt2[b].reshape(NHT, 128).T
    put("vt2g", v2.reshape(128, NHT * BL * GB))
    put("idh", np.eye(128, dtype=np.float16))
    put("colm", np.broadcast_to(np.eye(BL, dtype=np.float16).reshape(1, BL * BL),
                                (128, BL * BL)))
    cm16 = np.hstack([np.eye(BL, dtype=np.float16),
                      np.zeros((BL, 16 - BL), dtype=np.float16)])
    put("cm16", np.broadcast_to(cm16.reshape(1, BL * 16), (128, BL * 16)))
    return wp


def _packG(f):
    # transposed gate biases, broadcast along the b (free) axis:
    # bhhT [128, 12, GB] then bihT [128, 12, BL]
    wp = np.zeros((128, 12 * (GB + BL)), dtype=np.float16)
    bhh = f["gru_bhh"].astype(np.float16).reshape(12, 128).T
    bih = f["gru_bih"].astype(np.float16).reshape(12, 128).T
    wp[:, 0:12 * GB] = np.repeat(bhh[:, :, None], GB, axis=2).reshape(128, -1)
    wp[:, 12 * GB:] = np.repeat(bih[:, :, None], BL, axis=2).reshape(128, -1)
    return wp


def _packQ(qe):
    wp = np.zeros((128, WQTOT), dtype=np.float16)
    o, ln = WQ["qeT"]
    qeT = np.ascontiguousarray(qe.transpose(2, 1, 0)).astype(np.float16)
    wp[:, o:o + ln] = _tiles(qeT.reshape(D, BL * LQ), NKT)
    return wp


def _packB(f):
    # x16 lifts the ~N(0, 0.05^2) weights out of fp8's subnormal range;
    # compensated on-chip (gi: x1/16 in the bias add; gh: cancels the
    # ct fp8 copy's 1/16 pre-scale)
    wp = np.zeros((128, WBTOT), dtype=np.float32)
    o, ln = WB["wihT"]
    wp[:, o:o + ln] = _tiles(f["gru_wih"].T.astype(np.float32) * 16.0, NKT)
    o, ln = WB["whhT"]
    wp[:, o:o + ln] = _tiles(f["gru_whh"].T.astype(np.float32) * 16.0, NKT)
    return _fp8(wp)


def _pack32(f):
    wp = np.zeros((128, W32TOT), dtype=np.float32)
    o, ln = W32["idf"]
    wp[:, o:o + ln] = np.eye(128, dtype=np.float32)
    o, ln = W32["cqb"]
    wp[:, o:o + ln] = (f["WQu_b"] + f["WQv_b"]).astype(np.float32).reshape(NHT, 128).T
    o, ln = W32["wb"]
    wp[:, o:o + ln] = (f["WPh_b"] + f["Wah_b"]).astype(np.float32).reshape(NHT, 128).T
    return wp


def _fp8(x):
    import ml_dtypes
    return np.ascontiguousarray(x).astype(ml_dtypes.float8_e4m3).view(np.uint8)


def make_in_maps(f):
    passEnc, quesEnc = f["passEnc"], f["quesEnc"]
    wp32 = _pack32(f)
    wpB = _packB(f)
    in_maps = []
    for i in range(NC):
        s = slice(i * BL, (i + 1) * BL)
        pe = passEnc[:, s, :]
        qe = quesEnc[:, s, :]
        wpAfull = _packA(f, f["Vt1"][s, :, 0], f["Vt2"][s, :, 0])
        wpQ_ = _packQ(qe)
        # peC [g, pc, part, b', kt, d]: per-partition runs of 8KB
        peC = np.ascontiguousarray(
            pe.astype(np.float16).reshape(NPC, 512, NG, GB, NKT, 128).transpose(
                2, 0, 5, 3, 4, 1))
        # peN8 [g, j, part, i, b', sub, d]: global p = (2j+i)*256 + sub*128
        # + part; per-partition contiguous runs of 4KB
        peN8 = _fp8(pe.reshape(NPR // 2, 2, 2, 128, NG, GB, D).transpose(
            4, 0, 3, 1, 5, 2, 6))
        in_maps.append({
            "peC": peC,
            "peN8": peN8,
            "wpA": np.ascontiguousarray(wpAfull[:, :WAHOT]),
            "wpA2": np.ascontiguousarray(wpAfull[:, WAHOT:]),
            "wpQ": wpQ_, "wpB": wpB, "wp32": wp32,
            "wpN": qe.astype(np.float16).reshape(LQ, BL * D),
            "wpG": _packG(f),
        })
    return in_maps


def kernel(**inputs):
    f = {k: np.asarray(v) for k, v in inputs.items()}
    in_maps = make_in_maps(f)
    nc = _get_nc()
    res = run_bass_kernel_spmd(nc, in_maps, core_ids=list(range(NC)))
    aP1 = np.concatenate([res.results[i]["out"][0] for i in range(NC)], axis=0)
    aP2 = np.concatenate([res.results[i]["out"][1] for i in range(NC)], axis=0)
    return (aP1.astype(np.float32), aP2.astype(np.float32))


# revision 44
# speedup vs baseline: 1.1857x; 1.0700x over previous
"""Answer-pointer network forward pass on 8 TRN2 NeuronCores.

Data-parallel over batch: B=64 -> 8 batches per core, weights replicated.
No collectives; each core computes softmax attention maps (aP1, aP2) for
its batch shard and the host concatenates.

Schedule: the 8 per-core batches are split into G=4 groups of GB=2 and
software-pipelined. Nothing couples batches mathematically (softmax, ct,
GRU are all per-batch), so group g's pointer-step-2 (scalar-engine-bound:
tanh over [LP, GB, H]) runs concurrently with group g+1's pointer-step-1
(tensor-engine-bound: the WPh linear over [LP, GB, 2H]). This hides the
~37us of step-2 tanh that a batch-monolithic schedule serializes after
the GRU, leaving only the last group's step-2 exposed at the tail.

Within a group, P1 is chunk-pipelined over 4 p-chunks of 512: passP
matmuls (tensor) -> PSUM->SBUF copies (vector/gpsimd) -> tanh (scalar)
-> masked sP matmuls (tensor) -> per-chunk exp with Z-accumulation
(scalar). sP/exp for chunk pc are emitted one iteration later than the
chunk's tanh so no engine queue ever heads into a not-yet-satisfied
dependency (engines execute their queues in order; a stalled head op
blocks ready work behind it).

Layouts (host-side prep, outside HW exec):
  - peC  [pc, b, 128, kt, 512] fp16: passEnc feature-major for the WPh
    linear (contract over d=512 on partitions).
  - peN8 [pr, 128, b, 2, 512] fp8e4: passEnc position-major for the
    attention-weighted context ct, pre-packed for fp8 DoubleRow matmuls
    (contract over p=2048; each DR instruction consumes K=256).

Measured PE cost law: a matmul instruction costs ~N_out_columns x
0.417ns at full p-state; the tensor engine clock ramps with sustained
use (idle gaps drop it to ~0.83ns/col for up to 3us), so the schedule
aims to keep the PE continuously fed.

The GRU consumes the *unnormalized* context sum (matmul is linear) so
its matmuls start before the softmax normalizer 1/Z is ready; 1/Z is
folded into the gate bias-add. Sigmoid is computed as 0.5*tanh(x/2)+0.5
so every activation in the kernel (tanh/exp/identity) lives in the one
'exp_and_others' table -- zero ACT_TABLE_LOAD switches after the first.

Per-batch reductions (sP, sQ, rQ, ct) use masked stationary operands:
column (b mod GB) of the lhsT kept, rest zeroed, so batch b's matmul
writes only PSUM row (b mod GB); accumulating over the group assembles
[GB, N] without partition-offset copies.
"""

import numpy as np

try:
    import concourse.bass as bass
except ImportError:  # pragma: no cover
    import sys

    sys.path.insert(0, "/opt/trn_rl_repo")
    import concourse.bass as bass

import concourse.tile as tile
from concourse import bacc, mybir
from concourse.bass_utils import run_bass_kernel_spmd

F8 = mybir.dt.float8e4
F16 = mybir.dt.float16
F32 = mybir.dt.float32
AF = mybir.ActivationFunctionType
OP = mybir.AluOpType
DR = mybir.MatmulPerfMode.DoubleRow

H = 256      # hidden
D = 512      # 2*hidden
LP = 2048    # passage length
LQ = 64      # question length
B = 64       # global batch
BL = 8       # batch per core
G6 = 6 * H   # 1536, GRU gate width
NC = 8       # cores
NKT = D // 128    # 4 contraction tiles over d
NHT = H // 128    # 2 tiles over h
NPC = LP // 512   # 4 p-chunks of 512
NPR = LP // 256   # 8 p-pairs of 256 (DoubleRow K tiles)
NG = 4            # batch groups per core
GB = BL // NG     # 2 batches per group


def _layout(entries):
    off, table = 0, {}
    for name, ln in entries:
        table[name] = (off, ln)
        off += ln
    return table, off


# hot entries (Q-phase) first: they arrive in a separate, earlier DMA
WA, WATOT = _layout([
    ("WQvT", NHT * H), ("WQuT", NKT * H), ("VQrT", NHT),
    ("vt1m", NHT * BL * BL),
    ("idh", 128), ("colm", BL * BL), ("cm16", BL * 16),
    ("WPhT", NKT * H), ("WahT", NKT * H), ("vt2g", NHT * BL * GB),
])
WAHOT = WA["idh"][0]   # first four entries form the hot prefix
WQ, WQTOT = _layout([("qeT", NKT * BL * LQ)])
# GRU weight matrices in fp8 (inputs rq1/ct are small weighted averages;
# quantization noise lands ~5e-4 on the final softmax)
WB, WBTOT = _layout([("wihT", NKT * G6), ("whhT", NKT * G6)])
W32, W32TOT = _layout([("idf", 128), ("cqb", NHT), ("wb", NHT)])

_CACHED_NC = None


def _build():
    nc = bacc.Bacc("TRN2", target_bir_lowering=False, debug=False, num_devices=NC)

    peC = nc.dram_tensor("peC", [NG, NPC, 128, GB, NKT, 512], F16,
                         kind="ExternalInput").ap()
    peN8 = nc.dram_tensor("peN8", [NG, NPR // 2, 128, 2, GB, 2, 512], F8,
                         kind="ExternalInput").ap()
    wpA = nc.dram_tensor("wpA", [128, WAHOT], F16, kind="ExternalInput").ap()
    wpA2 = nc.dram_tensor("wpA2", [128, WATOT - WAHOT], F16,
                          kind="ExternalInput").ap()
    wpQ = nc.dram_tensor("wpQ", [128, WQTOT], F16, kind="ExternalInput").ap()
    wpN = nc.dram_tensor("wpN", [LQ, BL * D], F16, kind="ExternalInput").ap()
    wpG = nc.dram_tensor("wpG", [128, 12 * (GB + BL)], F16,
                         kind="ExternalInput").ap()
    wpB = nc.dram_tensor("wpB", [128, WBTOT], F8, kind="ExternalInput").ap()
    wp32 = nc.dram_tensor("wp32", [128, W32TOT], F32, kind="ExternalInput").ap()
    out = nc.dram_tensor("out", [2, BL, LP], F32, kind="ExternalOutput").ap()

    with tile.TileContext(nc) as tc:
        sing = tc.alloc_tile_pool(name="sing", bufs=1)

        def _single(shape, dtype, name):
            return sing.tile(shape, dtype, name=name, tag=name)

        chunkp = tc.alloc_tile_pool(name="chunk", bufs=3)
        pnp = tc.alloc_tile_pool(name="pn", bufs=4)
        t2p = tc.alloc_tile_pool(name="t2", bufs=3)
        t2bp = tc.alloc_tile_pool(name="t2b", bufs=3)
        apb = tc.alloc_tile_pool(name="apb", bufs=2)
        wmp = tc.alloc_tile_pool(name="wm", bufs=2)
        grup = tc.alloc_tile_pool(name="gru", bufs=1)
        # PSUM budget: ppp 4 banks + rowps 3 + ctps 1 = 8
        ppp = tc.alloc_tile_pool(name="ppp", bufs=4, space="PSUM")
        rowps = tc.alloc_tile_pool(name="rowps", bufs=3, space="PSUM")
        ctps = tc.alloc_tile_pool(name="ctps", bufs=1, space="PSUM")

        # ---- packed weights, hot-first ----
        wpA_s = _single([128, WATOT], F16, "wpA_s")
        nc.sync.dma_start(wpA_s[:, 0:WAHOT], wpA)
        pe_tiles = {}
        pn_tiles = {}

        def fetch_peC(c):
            g, pc = divmod(c, NPC)
            t = chunkp.tile([128, GB, NKT, 512], F16, tag="pe", name=f"pe{c}")
            eng = nc.sync if c % 2 == 0 else nc.gpsimd
            eng.dma_start(t, peC[g, pc])
            pe_tiles[(g, pc)] = t

        def fetch_pn(g, j):
            t = pnp.tile([128, 2, GB, 2, 512], F8, tag="pn", name=f"pn{g}_{j}")
            nc.sync.dma_start(t, peN8[g, j])
            pn_tiles[(g, j)] = t

        wpQ_s = _single([128, WQTOT], F16, "wpQ_s")
        nc.sync.dma_start(wpQ_s, wpQ)
        wp32_s = _single([128, W32TOT], F32, "wp32_s")
        nc.sync.dma_start(wp32_s, wp32)
        nc.sync.dma_start(wpA_s[:, WAHOT:WATOT], wpA2)
        fetch_peC(0)
        wpN_s = _single([LQ, BL * D], F16, "wpN_s")
        nc.gpsimd.dma_start(wpN_s, wpN)
        fetch_peC(1)
        wpB_s = _single([128, WBTOT], F8, "wpB_s")
        nc.gpsimd.dma_start(wpB_s, wpB)
        wpG_s = _single([128, 12 * (GB + BL)], F16, "wpG_s")
        nc.gpsimd.dma_start(wpG_s, wpG)

        def sA(name):
            o, ln = WA[name]
            return wpA_s[:, o:o + ln]

        WQvT_s = sA("WQvT").rearrange("p (kt h) -> p kt h", kt=NHT)
        WQuT_s = sA("WQuT").rearrange("p (kt h) -> p kt h", kt=NKT)
        WPhT_s = sA("WPhT").rearrange("p (kt h) -> p kt h", kt=NKT)
        WahT_s = sA("WahT").rearrange("p (kt h) -> p kt h", kt=NKT)
        VQrT_s = sA("VQrT").rearrange("p (ht o) -> p ht o", ht=NHT)
        vt1m_s = sA("vt1m").rearrange("p (ht b c) -> p ht b c", ht=NHT, b=BL)
        vt2g_s = sA("vt2g").rearrange("p (ht b c) -> p ht b c", ht=NHT, b=BL)
        idh_s = sA("idh")
        colm_s = sA("colm").rearrange("p (b c) -> p b c", b=BL)
        cm16_s = sA("cm16").rearrange("p (b m) -> p b m", b=BL)
        qeT_s = wpQ_s[:, WQ["qeT"][0]:WQ["qeT"][0] + NKT * BL * LQ].rearrange(
            "p (kt bq) -> p kt bq", kt=NKT)
        qeN_s = wpN_s[:, :]
        wihT_s = wpB_s[:, WB["wihT"][0]:WB["wihT"][0] + NKT * G6].rearrange(
            "p (pr sub g) -> p pr sub g", pr=NKT // 2, sub=2)
        whhT_s = wpB_s[:, WB["whhT"][0]:WB["whhT"][0] + NKT * G6].rearrange(
            "p (pr sub g) -> p pr sub g", pr=NKT // 2, sub=2)
        bhhT_s = wpG_s[:, 0:12 * GB].rearrange("p (gt c) -> p gt c", gt=12)
        bihT_s = wpG_s[:, 12 * GB:12 * (GB + BL)].rearrange(
            "p (gt c) -> p gt c", gt=12)
        idf_s = wp32_s[:, W32["idf"][0]:W32["idf"][0] + 128]
        cqb_s = wp32_s[:, W32["cqb"][0]:W32["cqb"][0] + NHT]
        wb_s = wp32_s[:, W32["wb"][0]:W32["wb"][0] + NHT].rearrange(
            "p (ht o) -> p ht o", ht=NHT)

        # persistent activations
        ppr_s = _single([128, NHT, BL, NPC, 512], F16, "ppr_s")  # raw passP
        biasP_s = _single([128, 2, NHT, BL], F32, "biasP_s")
        # per-group exp(sP) rows at base partition 0 (matmul/transpose
        # inputs must start at partition 0/32/64)
        w1p = tc.alloc_tile_pool(name="w1p", bufs=2)
        w2p = tc.alloc_tile_pool(name="w2p", bufs=2)
        zlp = tc.alloc_tile_pool(name="zlp", bufs=2)
        w1_t, w2_t = {}, {}
        # engine accesses must start at partition 0 (mult-of-32), so all
        # per-group [GB, ...] data lives in its own base-0 tile
        zpart_t, zp2_t, z1_t, rz1_t, z2_t, rz2_t = ({} for _ in range(6))
        rq1_s = _single([BL, D], F32, "rq1_s")
        rq1T_s = [_single([128, BL], F16, f"rq1T{k}") for k in range(NKT)]
        rq1T8_s = [_single([128, 2, 16], F8, f"rq1T8{k}") for k in range(NKT // 2)]
        giT_s = _single([128, 12, BL], F16, "giT_s")
        colm8_s = _single([128, BL, 16], F8, "colm8_s")
        nc.vector.tensor_copy(colm8_s, cm16_s)

        def bcast_dim(ap, axis, size):
            """Insert a stride-0 (broadcast) free dim at position axis."""
            entries = list(ap.ap)
            entries.insert(axis, [0, size])
            return bass.AP(tensor=ap.tensor, offset=ap.offset, ap=entries)

        # ================= pipelined P1 / P2 stages =================
        t1saved, t2saved, sp1_ps, sp2_ps = {}, {}, {}, {}

        def em_a(g, pc, scalar_copies=False):
            """passP matmuls for group g, chunk pc; copies PSUM -> ppr.
            At group boundaries the copies go on the scalar engine (idle
            while waiting for the GRU gate chain) so the vector queue
            stays clear for that chain."""
            pe = pe_tiles.pop((g, pc))
            for b in range(GB):
                pps = [ppp.tile([128, 512], F32, tag="acc", name=f"pp{ht}")
                       for ht in range(NHT)]
                for kt in range(NKT):
                    for ht in range(NHT):
                        nc.tensor.matmul(pps[ht],
                                         lhsT=WPhT_s[:, kt, ht * 128:(ht + 1) * 128],
                                         rhs=pe[:, b, kt, :],
                                         start=kt == 0, stop=kt == NKT - 1)
                for ht in range(NHT):
                    dst = ppr_s[:, ht, GB * g + b, pc, :]
                    if scalar_copies:
                        nc.scalar.activation(dst, pps[ht], AF.Copy)
                    else:
                        nc.vector.tensor_copy(dst, pps[ht])

        def em_b_tanh(g, pc):
            for b in range(GB):
                t2 = t2p.tile([128, NHT, 512], F16, tag="t2", name="t2")
                for ht in range(NHT):
                    nc.scalar.activation(t2[:, ht, :],
                                         ppr_s[:, ht, GB * g + b, pc, :], AF.Tanh,
                                         bias=biasP_s[:, 0, ht,
                                                      GB * g + b:GB * g + b + 1],
                                         scale=1.0)
                t1saved[(g, pc, b)] = t2

        def em_sp1(g, pc):
            ps = rowps.tile([GB, 512], F32, tag="row", name=f"sp1_{pc}")
            for b in range(GB):
                t2 = t1saved.pop((g, pc, b))
                for ht in range(NHT):
                    nc.tensor.matmul(ps, lhsT=vt2g_s[:, ht, GB * g + b, :],
                                     rhs=t2[:, ht, :],
                                     start=(b == 0 and ht == 0),
                                     stop=(b == GB - 1 and ht == NHT - 1))
            sp1_ps[(g, pc)] = ps

        def em_exp1(g, pc):
            ps = sp1_ps.pop((g, pc))
            if pc == 0:
                w1_t[g] = w1p.tile([GB, LP], F16, tag="w1", name=f"w1_{g}")
                zpart_t[g] = zlp.tile([GB, NPC], F32, tag="zp1", name=f"zp1_{g}")
            nc.scalar.activation(w1_t[g][:, pc * 512:(pc + 1) * 512], ps, AF.Exp,
                                 accum_out=zpart_t[g][:, pc:pc + 1])

        def em_p2_tanh(g, pc):
            for b in range(GB):
                t2 = t2bp.tile([128, NHT, 512], F16, tag="t2b", name="t2b")
                for ht in range(NHT):
                    nc.scalar.activation(t2[:, ht, :],
                                         ppr_s[:, ht, GB * g + b, pc, :], AF.Tanh,
                                         bias=biasP_s[:, 1, ht,
                                                      GB * g + b:GB * g + b + 1],
                                         scale=1.0)
                t2saved[(g, pc, b)] = t2

        def em_p2_mm(g, pc):
            ps = rowps.tile([GB, 512], F32, tag="row", name=f"sp2_{pc}")
            for b in range(GB):
                t2 = t2saved.pop((g, pc, b))
                for ht in range(NHT):
                    nc.tensor.matmul(ps, lhsT=vt2g_s[:, ht, GB * g + b, :],
                                     rhs=t2[:, ht, :],
                                     start=(b == 0 and ht == 0),
                                     stop=(b == GB - 1 and ht == NHT - 1))
            sp2_ps[(g, pc)] = ps

        def em_p2_exp(g, pc):
            ps = sp2_ps.pop((g, pc))
            if pc == 0:
                w2_t[g] = w2p.tile([GB, LP], F16, tag="w2", name=f"w2_{g}")
                zp2_t[g] = zlp.tile([GB, NPC], F32, tag="zp2", name=f"zp2_{g}")
            nc.scalar.activation(w2_t[g][:, pc * 512:(pc + 1) * 512], ps, AF.Exp,
                                 accum_out=zp2_t[g][:, pc:pc + 1])

        def finish_p2(g):
            rows = slice(GB * g, GB * (g + 1))
            z2 = zlp.tile([GB, 1], F32, tag="z2", name=f"z2_{g}")
            rz2 = zlp.tile([GB, 1], F32, tag="rz2", name=f"rz2_{g}")
            nc.vector.reduce_sum(z2, zp2_t.pop(g), axis=mybir.AxisListType.X)
            nc.vector.reciprocal(rz2, z2)
            w2g = w2_t.pop(g)
            for h in range(2):
                hs = slice(h * LP // 2, (h + 1) * LP // 2)
                apc = apb.tile([GB, LP // 2], F32, tag="ap", name="ap2c")
                nc.vector.tensor_scalar(apc, w2g[:, hs], rz2, None,
                                        op0=OP.mult)
                nc.sync.dma_start(out=out[1, rows, hs], in_=apc)

        # ================= group epilogue: softmax Z, ct, GRU =============
        grust = {}

        def gru_a(g, ps_ct):
            """ct normalize + transpose + ghT matmuls. 1/Z is folded into
            the ct PSUM->SBUF copy (per-partition scale in [GB, D] layout),
            so everything downstream uses normalized ct."""
            ctn = grup.tile([GB, D], F16, tag="ctn", name=f"ctn{g}")
            nc.vector.tensor_scalar(ctn, ps_ct, rz1_t[g], None, op0=OP.mult)
            ctT16 = grup.tile([128, NKT, GB], F16, tag="ctT16", name=f"ctT16{g}")
            ctT8 = [grup.tile([128, 2, 16], F8, tag=f"ctT8{j}", name=f"ctT8_{g}{j}")
                    for j in range(NKT // 2)]
            for kt in range(NKT):
                ps_t = ppp.tile([128, GB], F16, tag="acc", name="ps_t")
                nc.tensor.transpose(ps_t, ctn[:, kt * 128:(kt + 1) * 128],
                                    idh_s[:GB, :GB])
                nc.vector.tensor_copy(ctT16[:, kt, :], ps_t)
                nc.vector.tensor_copy(ctT8[kt // 2][:, kt % 2, 0:GB], ps_t)
            # ghT[gate, b] = (ct_norm @ whh.T).T / 16 + bhh  (weights are x16)
            ghT = grup.tile([128, 12, GB], F16, tag="ghT", name=f"ghT{g}")
            for gt in range(12):
                ps_g = rowps.tile([128, GB], F32, tag="row", name="ps_g")
                for p2 in range(NKT // 2):
                    nc.tensor.matmul(
                        ps_g, lhsT=whhT_s[:, p2, :, gt * 128:(gt + 1) * 128],
                        rhs=ctT8[p2][:, :, 0:GB],
                        start=p2 == 0, stop=p2 == NKT // 2 - 1, perf_mode=DR)
                nc.vector.tensor_scalar(ghT[:, gt, :], ps_g, 1.0 / 16.0, None,
                                        op0=OP.mult)
            nc.vector.tensor_add(ghT, ghT, bhhT_s)
            grust[g] = (ctT16, ghT)

        def gru_b(g):
            """gate elementwise chain in transposed layout ([128, <=12, GB]:
            free size <= 24 per op). r,z = sigmoid(gi+gh) computed as
            0.5*tanh(x/2)+0.5 (stays in the exp/tanh activation table ->
            no ACT_TABLE_LOAD switches). Gate rows: 0:4 = r, 4:8 = z,
            8:12 = n."""
            ctT16, ghT = grust[g]
            giTg = giT_s[:, :, slice(GB * g, GB * (g + 1))]
            rzin = grup.tile([128, 8, GB], F16, tag="rzin", name="rzin")
            nc.vector.tensor_add(rzin, giTg[:, 0:8, :], ghT[:, 0:8, :])
            th = grup.tile([128, 8, GB], F16, tag="th", name="th")
            nc.scalar.activation(th, rzin, AF.Tanh, scale=0.5)
            # n = tanh(gi_n + r*gh_n); r*gh_n = 0.5*(th_r*gh_n + gh_n)
            a_t = grup.tile([128, 4, GB], F16, tag="ga", name="ga")
            nc.vector.tensor_mul(a_t, th[:, 0:4, :], ghT[:, 8:12, :])
            nc.vector.tensor_add(a_t, a_t, ghT[:, 8:12, :])
            nin = grup.tile([128, 4, GB], F16, tag="nin", name="nin")
            nc.vector.scalar_tensor_tensor(nin, a_t, 0.5, giTg[:, 8:12, :],
                                           op0=OP.mult, op1=OP.add)
            n_t = grup.tile([128, 4, GB], F16, tag="gn", name="gn")
            nc.scalar.activation(n_t, nin, AF.Tanh)
            # h' = n + z*(ct-n); z*(ct-n) = 0.5*(th_z*d + d), d = ctT - n
            d_t = grup.tile([128, 4, GB], F16, tag="gd", name="gd")
            nc.vector.tensor_sub(d_t, ctT16, n_t)
            e_t = grup.tile([128, 4, GB], F16, tag="ge", name="ge")
            nc.vector.tensor_mul(e_t, th[:, 4:8, :], d_t)
            nc.vector.tensor_add(e_t, e_t, d_t)
            rq2T = grup.tile([128, 4, GB], F16, tag="rq2T", name="rq2T")
            nc.vector.scalar_tensor_tensor(rq2T, e_t, 0.5, n_t,
                                           op0=OP.mult, op1=OP.add)
            grust[g] = rq2T

        def gru_c(g):
            rq2T = grust.pop(g)
            rows = slice(GB * g, GB * (g + 1))
            for ht in range(NHT):
                ps_w = ppp.tile([128, GB], F32, tag="acc", name="ps_w")
                for kt in range(NKT):
                    nc.tensor.matmul(ps_w,
                                     lhsT=WahT_s[:, kt, ht * 128:(ht + 1) * 128],
                                     rhs=rq2T[:, kt, :], start=kt == 0,
                                     stop=kt == NKT - 1)
                nc.vector.tensor_scalar(biasP_s[:, 1, ht, rows], ps_w,
                                        wb_s[:, ht, :], None, op0=OP.add)

        def group_end(g):
            rows = slice(GB * g, GB * (g + 1))
            if g + 1 < NG:
                fetch_peC(NPC * (g + 1) + 2)
            # 1/Z for step 1: ready while the ct matmuls run
            z1_t[g] = zlp.tile([GB, 1], F32, tag="z1", name=f"z1_{g}")
            rz1_t[g] = zlp.tile([GB, 1], F32, tag="rz1", name=f"rz1_{g}")
            nc.vector.reduce_sum(z1_t[g], zpart_t.pop(g),
                                 axis=mybir.AxisListType.X)
            nc.vector.reciprocal(rz1_t[g], z1_t[g])
            # ct += w1[rows] @ passEnc, via fp8 DoubleRow
            ps_ct = ctps.tile([GB, D], F32, tag="ct", name=f"ct{g}")
            for j in range(NPR // 2):
                pn = pn_tiles.pop((g, j))
                for i in range(2):
                    pr = 2 * j + i
                    wm8 = wmp.tile([128, 2, GB, 16], F8, tag="wm", name="wm8")
                    for sub in range(2):
                        pt = pr * 2 + sub
                        ps_wt = ppp.tile([128, GB], F16, tag="acc", name="ps_wt")
                        nc.tensor.transpose(
                            ps_wt, w1_t[g][:, pt * 128:(pt + 1) * 128],
                            idh_s[:GB, :GB])
                        nc.vector.tensor_mul(wm8[:, sub],
                                             bcast_dim(ps_wt[:, :], 2, 16),
                                             colm8_s[:, 0:GB, :])
                    for b in range(GB):
                        nc.tensor.matmul(
                            ps_ct, lhsT=wm8[:, :, b, 0:GB], rhs=pn[:, i, b, :, :],
                            start=(pr == 0 and b == 0),
                            stop=(pr == NPR - 1 and b == GB - 1),
                            perf_mode=DR)
            # the GRU serial chain is interleaved with the next group's
            # first two passP blocks so the tensor queue never runs dry;
            # tensor order: stage_c -> em_a(0) -> ct transposes + ghT ->
            # em_a(1) (covers the gate elementwise chain) -> wah -> sp1
            if g + 1 < NG:
                fetch_pn(g + 1, 0)
                em_a(g + 1, 0, scalar_copies=True)
            gru_a(g, ps_ct)
            gru_b(g)
            if g + 1 < NG:
                fetch_peC(NPC * (g + 1) + 3)
                em_a(g + 1, 1)
            gru_c(g)
            if g + 1 < NG:
                em_b_tanh(g + 1, 0)
                em_sp1(g + 1, 0)
                em_exp1(g + 1, 0)
                em_b_tanh(g + 1, 1)
            # aP1 normalize + store: off the critical path
            w1g = w1_t.pop(g)
            for h in range(2):
                hs = slice(h * LP // 2, (h + 1) * LP // 2)
                apc = apb.tile([GB, LP // 2], F32, tag="ap", name="ap1c")
                nc.vector.tensor_scalar(apc, w1g[:, hs], rz1_t[g], None,
                                        op0=OP.mult)
                nc.gpsimd.dma_start(out=out[0, rows, hs], in_=apc)

        # ---- Q phase        # ---- Q phase: question-aware initial state rQ, all 8 batches ----
        tqT_s = _single([128, NHT, BL * LQ], F16, "tqT_s")
        cb_s = _single([128, NHT], F32, "cb_s")
        esq = _single([BL, LQ], F32, "esq")
        zq = _single([BL, 1], F32, "zq")
        rzq = _single([BL, 1], F32, "rzq")
        a_s = _single([BL, LQ], F16, "a_s")
        atm_s = _single([LQ, BL, BL], F16, "atm_s")

        def q1():
            ps_qv = ppp.tile([128, NHT], F32, tag="acc", name="ps_qv")
            for ht in range(NHT):
                for kt in range(NHT):
                    nc.tensor.matmul(ps_qv[:, ht:ht + 1],
                                     lhsT=WQvT_s[:, kt, ht * 128:(ht + 1) * 128],
                                     rhs=VQrT_s[:, kt, :], start=kt == 0,
                                     stop=kt == NHT - 1)
            nc.vector.tensor_add(cb_s, ps_qv, cqb_s)
            for ht in range(NHT):
                ps_tq = ppp.tile([128, 512], F32, tag="acc", name="ps_tq")
                for kt in range(NKT):
                    nc.tensor.matmul(ps_tq,
                                     lhsT=WQuT_s[:, kt, ht * 128:(ht + 1) * 128],
                                     rhs=qeT_s[:, kt, :], start=kt == 0,
                                     stop=kt == NKT - 1)
                nc.scalar.activation(tqT_s[:, ht, :], ps_tq, AF.Tanh,
                                     bias=cb_s[:, ht:ht + 1], scale=1.0)

        def q2():
            ps_sq = rowps.tile([BL, LQ], F32, tag="row", name="ps_sq")
            for b in range(BL):
                for ht in range(NHT):
                    nc.tensor.matmul(ps_sq, lhsT=vt1m_s[:, ht, b, :],
                                     rhs=tqT_s[:, ht, b * LQ:(b + 1) * LQ],
                                     start=(b == 0 and ht == 0),
                                     stop=(b == BL - 1 and ht == NHT - 1))
            nc.scalar.activation(esq, ps_sq, AF.Exp, accum_out=zq)
            nc.vector.reciprocal(rzq, zq)
            nc.vector.tensor_scalar(a_s, esq, rzq, None, op0=OP.mult)

        def q3():
            ps_at = ppp.tile([LQ, BL], F16, tag="acc", name="ps_at")
            nc.tensor.transpose(ps_at, a_s, idh_s[:BL, :BL])
            nc.vector.tensor_mul(atm_s,
                                 bcast_dim(ps_at[:, :], 1, BL),
                                 colm_s[0:LQ, :, :])
            ps_rq = rowps.tile([BL, D], F32, tag="row", name="ps_rq")
            for b in range(BL):
                nc.tensor.matmul(ps_rq, lhsT=atm_s[:, b, :],
                                 rhs=qeN_s[:, b * D:(b + 1) * D],
                                 start=b == 0, stop=b == BL - 1)
            nc.vector.tensor_copy(rq1_s, ps_rq)

        def q4():
            for kt in range(NKT):
                ps_t = ppp.tile([128, BL], F32, tag="acc", name="ps_q4")
                nc.tensor.transpose(ps_t, rq1_s[:, kt * 128:(kt + 1) * 128],
                                    idf_s[:BL, :BL])
                nc.vector.tensor_copy(rq1T_s[kt], ps_t)
                nc.vector.tensor_copy(rq1T8_s[kt // 2][:, kt % 2, 0:BL], ps_t)
            for ht in range(NHT):
                ps_w = ppp.tile([128, BL], F32, tag="acc", name="ps_w0")
                for kt in range(NKT):
                    nc.tensor.matmul(ps_w,
                                     lhsT=WahT_s[:, kt, ht * 128:(ht + 1) * 128],
                                     rhs=rq1T_s[kt], start=kt == 0,
                                     stop=kt == NKT - 1)
                nc.vector.tensor_scalar(biasP_s[:, 0, ht, :], ps_w,
                                        wb_s[:, ht, :], None, op0=OP.add)

        def emit_gi():
            # giT[gate, b] = (rq1 @ wih.T).T / 16 + bih, fp8 DoubleRow,
            # all 8 batches at once in transposed layout
            for gt in range(12):
                ps_gi = rowps.tile([128, BL], F32, tag="row", name="ps_gi")
                for pr in range(NKT // 2):
                    nc.tensor.matmul(
                        ps_gi, lhsT=wihT_s[:, pr, :, gt * 128:(gt + 1) * 128],
                        rhs=rq1T8_s[pr][:, :, 0:BL],
                        start=pr == 0, stop=pr == NKT // 2 - 1, perf_mode=DR)
                nc.vector.tensor_scalar(giT_s[:, gt, :], ps_gi, 1.0 / 16.0,
                                        None, op0=OP.mult)
            nc.vector.tensor_add(giT_s, giT_s, bihT_s)

        # ================= emission =================
        q1()
        fetch_peC(2)
        fetch_pn(0, 0)
        em_a(0, 0)
        q2()
        fetch_peC(3)
        fetch_pn(0, 1)
        em_a(0, 1)
        q3()
        fetch_peC(4)
        fetch_pn(0, 2)
        em_a(0, 2)
        q4()
        em_b_tanh(0, 0)
        fetch_peC(5)
        fetch_pn(0, 3)
        em_a(0, 3)
        em_sp1(0, 0)
        em_exp1(0, 0)
        em_b_tanh(0, 1)
        em_sp1(0, 1)
        em_exp1(0, 1)
        em_b_tanh(0, 2)
        em_sp1(0, 2)
        em_exp1(0, 2)
        em_b_tanh(0, 3)
        em_sp1(0, 3)
        em_exp1(0, 3)
        emit_gi()
        group_end(0)

        for g in range(1, NG):
            # P2(g-1, 0) mini-iteration; em_a/tanh1/sp1 for (g, 0..1) were
            # emitted inside group_end(g-1), interleaved with the GRU
            fetch_pn(g, 1)
            em_p2_tanh(g - 1, 0)
            em_p2_mm(g - 1, 0)
            em_p2_exp(g - 1, 0)
            for pc in (2, 3):
                c = NPC * g + pc
                if c + 2 < NG * NPC:
                    fetch_peC(c + 2)
                fetch_pn(g, pc)
                em_sp1(g, pc - 1)
                em_exp1(g, pc - 1)
                em_a(g, pc)
                em_p2_tanh(g - 1, pc - 1)
                em_p2_mm(g - 1, pc - 1)
                em_b_tanh(g, pc)
                em_p2_exp(g - 1, pc - 1)
            em_sp1(g, NPC - 1)
            em_exp1(g, NPC - 1)
            em_p2_tanh(g - 1, NPC - 1)
            em_p2_mm(g - 1, NPC - 1)
            em_p2_exp(g - 1, NPC - 1)
            finish_p2(g - 1)
            group_end(g)

        # tail: last group's pointer step 2
        for pc in range(NPC):
            em_p2_tanh(NG - 1, pc)
            em_p2_mm(NG - 1, pc)
            em_p2_exp(NG - 1, pc)
        finish_p2(NG - 1)

        zlp.release()
        w2p.release()
        w1p.release()
        ctps.release()
        rowps.release()
        ppp.release()
        grup.release()
        wmp.release()
        apb.release()
        t2bp.release()
        t2p.release()
        pnp.release()
        chunkp.release()
        sing.release()

    nc.compile()
    return nc


def _get_nc():
    global _CACHED_NC
    if _CACHED_NC is None:
        _CACHED_NC = _build()
    return _CACHED_NC


def _tiles(mat, nkt):  # [nkt*128, X] -> [128, nkt*X]
    x = mat.shape[1]
    return np.ascontiguousarray(
        mat.reshape(nkt, 128, x).transpose(1, 0, 2).reshape(128, nkt * x))


def _packA(f, Vt1, Vt2):
    # Vt1, Vt2: [BL, H] for this core's batch slice
    wp = np.zeros((128, WATOT), dtype=np.float16)

    def put(name, arr):
        o, ln = WA[name]
        assert arr.shape[1] == ln, (name, arr.shape, ln)
        wp[:arr.shape[0], o:o + ln] = arr

    put("WQvT", _tiles(f["WQv_W"].T.astype(np.float16), NHT))
    put("WQuT", _tiles(f["WQu_W"].T.astype(np.float16), NKT))
    put("WPhT", _tiles(f["WPh_W"].T.astype(np.float16), NKT))
    put("WahT", _tiles(f["Wah_W"].T.astype(np.float16), NKT))
    put("VQrT", _tiles(f["VQr"].reshape(1, H).T.astype(np.float16), NHT))
    # vt1m [128, ht, b, col]: col b = Vt1[b] per ht tile, rest zero
    v1 = np.zeros((128, NHT, BL, BL), dtype=np.float16)
    for b in range(BL):
        v1[:, :, b, b] = Vt1[b].reshape(NHT, 128).T
    put("vt1m", v1.reshape(128, NHT * BL * BL))
    # vt2g [128, ht, b, col]: col (b % GB) = Vt2[b], rest zero
    v2 = np.zeros((128, NHT, BL, GB), dtype=np.float16)
    for b in range(BL):
        v2[:, :, b, b % GB] = Vt2[b].reshape(NHT, 128).T
    put("vt2g", v2.reshape(128, NHT * BL * GB))
    put("idh", np.eye(128, dtype=np.float16))
    put("colm", np.broadcast_to(np.eye(BL, dtype=np.float16).reshape(1, BL * BL),
                                (128, BL * BL)))
    cm16 = np.hstack([np.eye(BL, dtype=np.float16),
                      np.zeros((BL, 16 - BL), dtype=np.float16)])
    put("cm16", np.broadcast_to(cm16.reshape(1, BL * 16), (128, BL * 16)))
    return wp


def _packG(f):
    # transposed gate biases, broadcast along the b (free) axis:
    # bhhT [128, 12, GB] then bihT [128, 12, BL]
    wp = np.zeros((128, 12 * (GB + BL)), dtype=np.float16)
    bhh = f["gru_bhh"].astype(np.float16).reshape(12, 128).T
    bih = f["gru_bih"].astype(np.float16).reshape(12, 128).T
    wp[:, 0:12 * GB] = np.repeat(bhh[:, :, None], GB, axis=2).reshape(128, -1)
    wp[:, 12 * GB:] = np.repeat(bih[:, :, None], BL, axis=2).reshape(128, -1)
    return wp


def _packQ(qe):
    wp = np.zeros((128, WQTOT), dtype=np.float16)
    o, ln = WQ["qeT"]
    qeT = np.ascontiguousarray(qe.transpose(2, 1, 0)).astype(np.float16)
    wp[:, o:o + ln] = _tiles(qeT.reshape(D, BL * LQ), NKT)
    return wp


def _packB(f):
    # x16 lifts the ~N(0, 0.05^2) weights out of fp8's subnormal range;
    # compensated on-chip (gi: x1/16 in the bias add; gh: cancels the
    # ct fp8 copy's 1/16 pre-scale)
    wp = np.zeros((128, WBTOT), dtype=np.float32)
    o, ln = WB["wihT"]
    wp[:, o:o + ln] = _tiles(f["gru_wih"].T.astype(np.float32) * 16.0, NKT)
    o, ln = WB["whhT"]
    wp[:, o:o + ln] = _tiles(f["gru_whh"].T.astype(np.float32) * 16.0, NKT)
    return _fp8(wp)


def _pack32(f):
    wp = np.zeros((128, W32TOT), dtype=np.float32)
    o, ln = W32["idf"]
    wp[:, o:o + ln] = np.eye(128, dtype=np.float32)
    o, ln = W32["cqb"]
    wp[:, o:o + ln] = (f["WQu_b"] + f["WQv_b"]).astype(np.float32).reshape(NHT, 128).T
    o, ln = W32["wb"]
    wp[:, o:o + ln] = (f["WPh_b"] + f["Wah_b"]).astype(np.float32).reshape(NHT, 128).T
    return wp


def _fp8(x):
    import ml_dtypes
    return np.ascontiguousarray(x).astype(ml_dtypes.float8_e4m3).view(np.uint8)


def make_in_maps(f):
    passEnc, quesEnc = f["passEnc"], f["quesEnc"]
    wp32 = _pack32(f)
    wpB = _packB(f)
    in_maps = []
    for i in range(NC):
        s = slice(i * BL, (i + 1) * BL)
        pe = passEnc[:, s, :]
        qe = quesEnc[:, s, :]
        wpAfull = _packA(f, f["Vt1"][s, :, 0], f["Vt2"][s, :, 0])
        wpQ_ = _packQ(qe)
        # peC [g, pc, part, b', kt, d]: per-partition runs of 8KB
        peC = np.ascontiguousarray(
            pe.astype(np.float16).reshape(NPC, 512, NG, GB, NKT, 128).transpose(
                2, 0, 5, 3, 4, 1))
        # peN8 [g, j, part, i, b', sub, d]: global p = (2j+i)*256 + sub*128
        # + part; per-partition contiguous runs of 4KB
        peN8 = _fp8(pe.reshape(NPR // 2, 2, 2, 128, NG, GB, D).transpose(
            4, 0, 3, 1, 5, 2, 6))
        in_maps.append({
            "peC": peC,
            "peN8": peN8,
            "wpA": np.ascontiguousarray(wpAfull[:, :WAHOT]),
            "wpA2": np.ascontiguousarray(wpAfull[:, WAHOT:]),
            "wpQ": wpQ_, "wpB": wpB, "wp32": wp32,
            "wpN": qe.astype(np.float16).reshape(LQ, BL * D),
            "wpG": _packG(f),
        })
    return in_maps


def kernel(**inputs):
    f = {k: np.asarray(v) for k, v in inputs.items()}
    in_maps = make_in_maps(f)
    nc = _get_nc()
    res = run_bass_kernel_spmd(nc, in_maps, core_ids=list(range(NC)))
    aP1 = np.concatenate([res.results[i]["out"][0] for i in range(NC)], axis=0)
    aP2 = np.concatenate([res.results[i]["out"][1] for i in range(NC)], axis=0)
    return (aP1.astype(np.float32), aP2.astype(np.float32))


# revision 47
# speedup vs baseline: 1.2004x; 1.0124x over previous
"""Answer-pointer network forward pass on 8 TRN2 NeuronCores.

Data-parallel over batch: B=64 -> 8 batches per core, weights replicated.
No collectives; each core computes softmax attention maps (aP1, aP2) for
its batch shard and the host concatenates.

Schedule: the 8 per-core batches are split into G=4 groups of GB=2 and
software-pipelined. Nothing couples batches mathematically (softmax, ct,
GRU are all per-batch), so group g's pointer-step-2 (scalar-engine-bound:
tanh over [LP, GB, H]) runs concurrently with group g+1's pointer-step-1
(tensor-engine-bound: the WPh linear over [LP, GB, 2H]). This hides the
~37us of step-2 tanh that a batch-monolithic schedule serializes after
the GRU, leaving only the last group's step-2 exposed at the tail.

Within a group, P1 is chunk-pipelined over 4 p-chunks of 512: passP
matmuls (tensor) -> PSUM->SBUF copies (vector/gpsimd) -> tanh (scalar)
-> masked sP matmuls (tensor) -> per-chunk exp with Z-accumulation
(scalar). sP/exp for chunk pc are emitted one iteration later than the
chunk's tanh so no engine queue ever heads into a not-yet-satisfied
dependency (engines execute their queues in order; a stalled head op
blocks ready work behind it).

Layouts (host-side prep, outside HW exec):
  - peC  [pc, b, 128, kt, 512] fp16: passEnc feature-major for the WPh
    linear (contract over d=512 on partitions).
  - peN8 [pr, 128, b, 2, 512] fp8e4: passEnc position-major for the
    attention-weighted context ct, pre-packed for fp8 DoubleRow matmuls
    (contract over p=2048; each DR instruction consumes K=256).

Measured PE cost law: a matmul instruction costs ~N_out_columns x
0.417ns at full p-state; the tensor engine clock ramps with sustained
use (idle gaps drop it to ~0.83ns/col for up to 3us), so the schedule
aims to keep the PE continuously fed.

The GRU consumes the *unnormalized* context sum (matmul is linear) so
its matmuls start before the softmax normalizer 1/Z is ready; 1/Z is
folded into the gate bias-add. Sigmoid is computed as 0.5*tanh(x/2)+0.5
so every activation in the kernel (tanh/exp/identity) lives in the one
'exp_and_others' table -- zero ACT_TABLE_LOAD switches after the first.

Per-batch reductions (sP, sQ, rQ, ct) use masked stationary operands:
column (b mod GB) of the lhsT kept, rest zeroed, so batch b's matmul
writes only PSUM row (b mod GB); accumulating over the group assembles
[GB, N] without partition-offset copies.
"""

import numpy as np

try:
    import concourse.bass as bass
except ImportError:  # pragma: no cover
    import sys

    sys.path.insert(0, "/opt/trn_rl_repo")
    import concourse.bass as bass

import concourse.tile as tile
from concourse import bacc, mybir
from concourse.bass_utils import run_bass_kernel_spmd

F8 = mybir.dt.float8e4
F16 = mybir.dt.float16
F32 = mybir.dt.float32
AF = mybir.ActivationFunctionType
OP = mybir.AluOpType
DR = mybir.MatmulPerfMode.DoubleRow

H = 256      # hidden
D = 512      # 2*hidden
LP = 2048    # passage length
LQ = 64      # question length
B = 64       # global batch
BL = 8       # batch per core
G6 = 6 * H   # 1536, GRU gate width
NC = 8       # cores
NKT = D // 128    # 4 contraction tiles over d
NHT = H // 128    # 2 tiles over h
NPC = LP // 512   # 4 p-chunks of 512
NPR = LP // 256   # 8 p-pairs of 256 (DoubleRow K tiles)
NG = 4            # batch groups per core
GB = BL // NG     # 2 batches per group


def _layout(entries):
    off, table = 0, {}
    for name, ln in entries:
        table[name] = (off, ln)
        off += ln
    return table, off


# hot entries (Q-phase) first: they arrive in a separate, earlier DMA
WA, WATOT = _layout([
    ("WQvT", NHT * H), ("WQuT", NKT * H), ("VQrT", NHT),
    ("vt1m", NHT * BL * BL),
    ("idh", 128), ("colm", BL * BL), ("cm16", BL * 16),
    ("WPhT", NKT * H), ("WahT", NKT * H), ("vt2g", NHT * BL * GB),
])
WAHOT = WA["idh"][0]   # first four entries form the hot prefix
WQ, WQTOT = _layout([("qeT", NKT * BL * LQ)])
# GRU weight matrices in fp8 (inputs rq1/ct are small weighted averages;
# quantization noise lands ~5e-4 on the final softmax)
WB, WBTOT = _layout([("wihT", NKT * G6), ("whhT", NKT * G6)])
W32, W32TOT = _layout([("idf", 128), ("cqb", NHT), ("wb", NHT),
                       ("bhhn", 4)])

_CACHED_NC = None


def _build():
    nc = bacc.Bacc("TRN2", target_bir_lowering=False, debug=False, num_devices=NC)

    peC = nc.dram_tensor("peC", [NG, NPC, 128, GB, NKT, 512], F16,
                         kind="ExternalInput").ap()
    peN8 = nc.dram_tensor("peN8", [NG, NPR // 2, 128, 2, GB, 2, 512], F8,
                         kind="ExternalInput").ap()
    wpA = nc.dram_tensor("wpA", [128, WAHOT], F16, kind="ExternalInput").ap()
    wpA2 = nc.dram_tensor("wpA2", [128, WATOT - WAHOT], F16,
                          kind="ExternalInput").ap()
    wpQ = nc.dram_tensor("wpQ", [128, WQTOT], F16, kind="ExternalInput").ap()
    wpN = nc.dram_tensor("wpN", [LQ, BL * D], F16, kind="ExternalInput").ap()
    wpG = nc.dram_tensor("wpG", [128, 4 + 12 * BL], F16,
                         kind="ExternalInput").ap()
    wpB = nc.dram_tensor("wpB", [128, WBTOT], F8, kind="ExternalInput").ap()
    wp32 = nc.dram_tensor("wp32", [128, W32TOT], F32, kind="ExternalInput").ap()
    out = nc.dram_tensor("out", [2, BL, LP], F32, kind="ExternalOutput").ap()

    with tile.TileContext(nc) as tc:
        sing = tc.alloc_tile_pool(name="sing", bufs=1)

        def _single(shape, dtype, name):
            return sing.tile(shape, dtype, name=name, tag=name)

        chunkp = tc.alloc_tile_pool(name="chunk", bufs=4)
        pnp = tc.alloc_tile_pool(name="pn", bufs=4)
        t2p = tc.alloc_tile_pool(name="t2", bufs=3)
        t2bp = tc.alloc_tile_pool(name="t2b", bufs=3)
        apb = tc.alloc_tile_pool(name="apb", bufs=2)
        wmp = tc.alloc_tile_pool(name="wm", bufs=2)
        grup = tc.alloc_tile_pool(name="gru", bufs=1)
        # PSUM budget: ppp 4 banks + rowps 3 + ctps 1 = 8
        ppp = tc.alloc_tile_pool(name="ppp", bufs=4, space="PSUM")
        rowps = tc.alloc_tile_pool(name="rowps", bufs=3, space="PSUM")
        ctps = tc.alloc_tile_pool(name="ctps", bufs=1, space="PSUM")

        # ---- packed weights, hot-first ----
        wpA_s = _single([128, WATOT], F16, "wpA_s")
        nc.sync.dma_start(wpA_s[:, 0:WAHOT], wpA)
        pe_tiles = {}
        pn_tiles = {}

        def fetch_peC(c):
            g, pc = divmod(c, NPC)
            t = chunkp.tile([128, GB, NKT, 512], F16, tag="pe", name=f"pe{c}")
            eng = nc.sync if c % 2 == 0 else nc.gpsimd
            eng.dma_start(t, peC[g, pc])
            pe_tiles[(g, pc)] = t

        def fetch_pn(g, j):
            t = pnp.tile([128, 2, GB, 2, 512], F8, tag="pn", name=f"pn{g}_{j}")
            nc.sync.dma_start(t, peN8[g, j])
            pn_tiles[(g, j)] = t

        wpQ_s = _single([128, WQTOT], F16, "wpQ_s")
        nc.sync.dma_start(wpQ_s, wpQ)
        wp32_s = _single([128, W32TOT], F32, "wp32_s")
        nc.sync.dma_start(wp32_s, wp32)
        nc.sync.dma_start(wpA_s[:, WAHOT:WATOT], wpA2)
        fetch_peC(0)
        wpN_s = _single([LQ, BL * D], F16, "wpN_s")
        nc.gpsimd.dma_start(wpN_s, wpN)
        fetch_peC(1)
        wpB_s = _single([128, WBTOT], F8, "wpB_s")
        nc.gpsimd.dma_start(wpB_s, wpB)
        wpG_s = _single([128, 4 + 12 * BL], F16, "wpG_s")
        nc.gpsimd.dma_start(wpG_s, wpG)

        def sA(name):
            o, ln = WA[name]
            return wpA_s[:, o:o + ln]

        WQvT_s = sA("WQvT").rearrange("p (kt h) -> p kt h", kt=NHT)
        WQuT_s = sA("WQuT").rearrange("p (kt h) -> p kt h", kt=NKT)
        WPhT_s = sA("WPhT").rearrange("p (kt h) -> p kt h", kt=NKT)
        WahT_s = sA("WahT").rearrange("p (kt h) -> p kt h", kt=NKT)
        VQrT_s = sA("VQrT").rearrange("p (ht o) -> p ht o", ht=NHT)
        vt1m_s = sA("vt1m").rearrange("p (ht b c) -> p ht b c", ht=NHT, b=BL)
        vt2g_s = sA("vt2g").rearrange("p (ht b c) -> p ht b c", ht=NHT, b=BL)
        idh_s = sA("idh")
        colm_s = sA("colm").rearrange("p (b c) -> p b c", b=BL)
        cm16_s = sA("cm16").rearrange("p (b m) -> p b m", b=BL)
        qeT_s = wpQ_s[:, WQ["qeT"][0]:WQ["qeT"][0] + NKT * BL * LQ].rearrange(
            "p (kt bq) -> p kt bq", kt=NKT)
        qeN_s = wpN_s[:, :]
        wihT_s = wpB_s[:, WB["wihT"][0]:WB["wihT"][0] + NKT * G6].rearrange(
            "p (pr sub g) -> p pr sub g", pr=NKT // 2, sub=2)
        whhT_s = wpB_s[:, WB["whhT"][0]:WB["whhT"][0] + NKT * G6].rearrange(
            "p (pr sub g) -> p pr sub g", pr=NKT // 2, sub=2)
        bhhn_s = wp32_s[:, W32["bhhn"][0]:W32["bhhn"][0] + 4]
        bihT_s = wpG_s[:, 4:4 + 12 * BL].rearrange(
            "p (gt c) -> p gt c", gt=12)
        idf_s = wp32_s[:, W32["idf"][0]:W32["idf"][0] + 128]
        cqb_s = wp32_s[:, W32["cqb"][0]:W32["cqb"][0] + NHT]
        wb_s = wp32_s[:, W32["wb"][0]:W32["wb"][0] + NHT].rearrange(
            "p (ht o) -> p ht o", ht=NHT)

        # persistent activations
        ppr_s = _single([128, NHT, BL, NPC, 512], F16, "ppr_s")  # raw passP
        biasP_s = _single([128, 2, NHT, BL], F32, "biasP_s")
        # per-group exp(sP) rows at base partition 0 (matmul/transpose
        # inputs must start at partition 0/32/64)
        w1p = tc.alloc_tile_pool(name="w1p", bufs=2)
        w2p = tc.alloc_tile_pool(name="w2p", bufs=2)
        zlp = tc.alloc_tile_pool(name="zlp", bufs=2)
        w1_t, w2_t = {}, {}
        # engine accesses must start at partition 0 (mult-of-32), so all
        # per-group [GB, ...] data lives in its own base-0 tile
        zpart_t, zp2_t, z1_t, rz1_t, z2_t, rz2_t = ({} for _ in range(6))
        rq1_s = _single([BL, D], F32, "rq1_s")
        rq1T_s = [_single([128, BL], F16, f"rq1T{k}") for k in range(NKT)]
        rq1T8_s = [_single([128, 2, 16], F8, f"rq1T8{k}") for k in range(NKT // 2)]
        giT_s = _single([128, 12, BL], F16, "giT_s")
        colm8_s = _single([128, BL, 16], F8, "colm8_s")
        nc.vector.tensor_copy(colm8_s, cm16_s)

        def bcast_dim(ap, axis, size):
            """Insert a stride-0 (broadcast) free dim at position axis."""
            entries = list(ap.ap)
            entries.insert(axis, [0, size])
            return bass.AP(tensor=ap.tensor, offset=ap.offset, ap=entries)

        # ================= pipelined P1 / P2 stages =================
        t1saved, t2saved, sp1_ps, sp2_ps = {}, {}, {}, {}

        def em_a(g, pc, scalar_copies=False):
            """passP matmuls for group g, chunk pc; copies PSUM -> ppr.
            At group boundaries the copies go on the scalar engine (idle
            while waiting for the GRU gate chain) so the vector queue
            stays clear for that chain."""
            pe = pe_tiles.pop((g, pc))
            for b in range(GB):
                pps = [ppp.tile([128, 512], F32, tag="acc", name=f"pp{ht}")
                       for ht in range(NHT)]
                for kt in range(NKT):
                    for ht in range(NHT):
                        nc.tensor.matmul(pps[ht],
                                         lhsT=WPhT_s[:, kt, ht * 128:(ht + 1) * 128],
                                         rhs=pe[:, b, kt, :],
                                         start=kt == 0, stop=kt == NKT - 1)
                for ht in range(NHT):
                    dst = ppr_s[:, ht, GB * g + b, pc, :]
                    if scalar_copies:
                        nc.scalar.activation(dst, pps[ht], AF.Copy)
                    else:
                        nc.vector.tensor_copy(dst, pps[ht])

        def em_b_tanh(g, pc):
            for b in range(GB):
                t2 = t2p.tile([128, NHT, 512], F16, tag="t2", name="t2")
                for ht in range(NHT):
                    nc.scalar.activation(t2[:, ht, :],
                                         ppr_s[:, ht, GB * g + b, pc, :], AF.Tanh,
                                         bias=biasP_s[:, 0, ht,
                                                      GB * g + b:GB * g + b + 1],
                                         scale=1.0)
                t1saved[(g, pc, b)] = t2

        def em_sp1(g, pc):
            ps = rowps.tile([GB, 512], F32, tag="row", name=f"sp1_{pc}")
            for b in range(GB):
                t2 = t1saved.pop((g, pc, b))
                for ht in range(NHT):
                    nc.tensor.matmul(ps, lhsT=vt2g_s[:, ht, GB * g + b, :],
                                     rhs=t2[:, ht, :],
                                     start=(b == 0 and ht == 0),
                                     stop=(b == GB - 1 and ht == NHT - 1))
            sp1_ps[(g, pc)] = ps

        def em_exp1(g, pc):
            ps = sp1_ps.pop((g, pc))
            if pc == 0:
                w1_t[g] = w1p.tile([GB, LP], F16, tag="w1", name=f"w1_{g}")
                zpart_t[g] = zlp.tile([GB, NPC], F32, tag="zp1", name=f"zp1_{g}")
            nc.scalar.activation(w1_t[g][:, pc * 512:(pc + 1) * 512], ps, AF.Exp,
                                 accum_out=zpart_t[g][:, pc:pc + 1])

        def em_p2_tanh(g, pc):
            for b in range(GB):
                t2 = t2bp.tile([128, NHT, 512], F16, tag="t2b", name="t2b")
                for ht in range(NHT):
                    nc.scalar.activation(t2[:, ht, :],
                                         ppr_s[:, ht, GB * g + b, pc, :], AF.Tanh,
                                         bias=biasP_s[:, 1, ht,
                                                      GB * g + b:GB * g + b + 1],
                                         scale=1.0)
                t2saved[(g, pc, b)] = t2

        def em_p2_mm(g, pc):
            ps = rowps.tile([GB, 512], F32, tag="row", name=f"sp2_{pc}")
            for b in range(GB):
                t2 = t2saved.pop((g, pc, b))
                for ht in range(NHT):
                    nc.tensor.matmul(ps, lhsT=vt2g_s[:, ht, GB * g + b, :],
                                     rhs=t2[:, ht, :],
                                     start=(b == 0 and ht == 0),
                                     stop=(b == GB - 1 and ht == NHT - 1))
            sp2_ps[(g, pc)] = ps

        def em_p2_exp(g, pc):
            ps = sp2_ps.pop((g, pc))
            if pc == 0:
                w2_t[g] = w2p.tile([GB, LP], F16, tag="w2", name=f"w2_{g}")
                zp2_t[g] = zlp.tile([GB, NPC], F32, tag="zp2", name=f"zp2_{g}")
            nc.scalar.activation(w2_t[g][:, pc * 512:(pc + 1) * 512], ps, AF.Exp,
                                 accum_out=zp2_t[g][:, pc:pc + 1])

        def finish_p2(g):
            rows = slice(GB * g, GB * (g + 1))
            z2 = zlp.tile([GB, 1], F32, tag="z2", name=f"z2_{g}")
            rz2 = zlp.tile([GB, 1], F32, tag="rz2", name=f"rz2_{g}")
            nc.vector.reduce_sum(z2, zp2_t.pop(g), axis=mybir.AxisListType.X)
            nc.vector.reciprocal(rz2, z2)
            w2g = w2_t.pop(g)
            for h in range(2):
                hs = slice(h * LP // 2, (h + 1) * LP // 2)
                apc = apb.tile([GB, LP // 2], F32, tag="ap", name="ap2c")
                nc.vector.tensor_scalar(apc, w2g[:, hs], rz2, None,
                                        op0=OP.mult)
                nc.sync.dma_start(out=out[1, rows, hs], in_=apc)

        # ================= group epilogue: softmax Z, ct, GRU =============
        grust = {}

        def gru_a(g, ps_ct):
            """ct normalize + transpose + ghT matmuls. 1/Z is folded into
            the ct PSUM->SBUF copy (per-partition scale in [GB, D] layout),
            so everything downstream uses normalized ct."""
            ctn = grup.tile([GB, D], F16, tag="ctn", name=f"ctn{g}")
            nc.vector.tensor_scalar(ctn, ps_ct, rz1_t[g], None, op0=OP.mult)
            ctT16 = grup.tile([128, NKT, GB], F16, tag="ctT16", name=f"ctT16{g}")
            ctT8 = [grup.tile([128, 2, 16], F8, tag=f"ctT8{j}", name=f"ctT8_{g}{j}")
                    for j in range(NKT // 2)]
            for kt in range(NKT):
                ps_t = ppp.tile([128, GB], F16, tag="acc", name="ps_t")
                nc.tensor.transpose(ps_t, ctn[:, kt * 128:(kt + 1) * 128],
                                    idh_s[:GB, :GB])
                nc.vector.tensor_copy(ctT16[:, kt, :], ps_t)
                nc.vector.tensor_copy(ctT8[kt // 2][:, kt % 2, 0:GB], ps_t)
            # ghT[gate, b] = (ct_norm @ whh.T).T / 16 (+ bhh_n on the n
            # rows; bhh for the r/z rows is pre-folded into giT host-side)
            ghT = grup.tile([128, 12, GB], F16, tag="ghT", name=f"ghT{g}")
            for gt in range(12):
                ps_g = rowps.tile([128, GB], F32, tag="row", name="ps_g")
                for p2 in range(NKT // 2):
                    nc.tensor.matmul(
                        ps_g, lhsT=whhT_s[:, p2, :, gt * 128:(gt + 1) * 128],
                        rhs=ctT8[p2][:, :, 0:GB],
                        start=p2 == 0, stop=p2 == NKT // 2 - 1, perf_mode=DR)
                if gt < 8:
                    nc.vector.tensor_scalar(ghT[:, gt, :], ps_g, 1.0 / 16.0,
                                            None, op0=OP.mult)
                else:
                    nc.vector.tensor_scalar(ghT[:, gt, :], ps_g, 1.0 / 16.0,
                                            bhhn_s[:, gt - 8:gt - 7],
                                            op0=OP.mult, op1=OP.add)
            grust[g] = (ctT16, ghT)

        def gru_b(g):
            """gate elementwise chain in transposed layout ([128, <=12, GB]:
            free size <= 24 per op). r,z = sigmoid(gi+gh) computed as
            0.5*tanh(x/2)+0.5 (stays in the exp/tanh activation table ->
            no ACT_TABLE_LOAD switches). Gate rows: 0:4 = r, 4:8 = z,
            8:12 = n."""
            ctT16, ghT = grust[g]
            giTg = giT_s[:, :, slice(GB * g, GB * (g + 1))]
            rzin = grup.tile([128, 8, GB], F16, tag="rzin", name="rzin")
            nc.vector.tensor_add(rzin, giTg[:, 0:8, :], ghT[:, 0:8, :])
            th = grup.tile([128, 8, GB], F16, tag="th", name="th")
            nc.scalar.activation(th, rzin, AF.Tanh, scale=0.5)
            # n = tanh(gi_n + r*gh_n); r*gh_n = 0.5*(th_r+1)*gh_n
            a_t = grup.tile([128, 4, GB], F16, tag="ga", name="ga")
            nc.vector.scalar_tensor_tensor(a_t, th[:, 0:4, :], 1.0,
                                           ghT[:, 8:12, :],
                                           op0=OP.add, op1=OP.mult)
            nin = grup.tile([128, 4, GB], F16, tag="nin", name="nin")
            nc.vector.scalar_tensor_tensor(nin, a_t, 0.5, giTg[:, 8:12, :],
                                           op0=OP.mult, op1=OP.add)
            n_t = grup.tile([128, 4, GB], F16, tag="gn", name="gn")
            nc.scalar.activation(n_t, nin, AF.Tanh)
            # h' = n + z*(ct-n); z*(ct-n) = 0.5*(th_z+1)*d, d = ctT - n
            d_t = grup.tile([128, 4, GB], F16, tag="gd", name="gd")
            nc.vector.tensor_sub(d_t, ctT16, n_t)
            e_t = grup.tile([128, 4, GB], F16, tag="ge", name="ge")
            nc.vector.scalar_tensor_tensor(e_t, th[:, 4:8, :], 1.0, d_t,
                                           op0=OP.add, op1=OP.mult)
            rq2T = grup.tile([128, 4, GB], F16, tag="rq2T", name="rq2T")
            nc.vector.scalar_tensor_tensor(rq2T, e_t, 0.5, n_t,
                                           op0=OP.mult, op1=OP.add)
            grust[g] = rq2T

        def gru_c(g):
            rq2T = grust.pop(g)
            rows = slice(GB * g, GB * (g + 1))
            for ht in range(NHT):
                ps_w = ppp.tile([128, GB], F32, tag="acc", name="ps_w")
                for kt in range(NKT):
                    nc.tensor.matmul(ps_w,
                                     lhsT=WahT_s[:, kt, ht * 128:(ht + 1) * 128],
                                     rhs=rq2T[:, kt, :], start=kt == 0,
                                     stop=kt == NKT - 1)
                nc.vector.tensor_scalar(biasP_s[:, 1, ht, rows], ps_w,
                                        wb_s[:, ht, :], None, op0=OP.add)

        def group_end(g):
            rows = slice(GB * g, GB * (g + 1))
            if g + 1 < NG:
                fetch_peC(NPC * (g + 1) + 2)
            # 1/Z for step 1: ready while the ct matmuls run
            z1_t[g] = zlp.tile([GB, 1], F32, tag="z1", name=f"z1_{g}")
            rz1_t[g] = zlp.tile([GB, 1], F32, tag="rz1", name=f"rz1_{g}")
            nc.vector.reduce_sum(z1_t[g], zpart_t.pop(g),
                                 axis=mybir.AxisListType.X)
            nc.vector.reciprocal(rz1_t[g], z1_t[g])
            # ct += w1[rows] @ passEnc, via fp8 DoubleRow
            ps_ct = ctps.tile([GB, D], F32, tag="ct", name=f"ct{g}")
            for j in range(NPR // 2):
                pn = pn_tiles.pop((g, j))
                for i in range(2):
                    pr = 2 * j + i
                    wm8 = wmp.tile([128, 2, GB, 16], F8, tag="wm", name="wm8")
                    for sub in range(2):
                        pt = pr * 2 + sub
                        ps_wt = ppp.tile([128, GB], F16, tag="acc", name="ps_wt")
                        nc.tensor.transpose(
                            ps_wt, w1_t[g][:, pt * 128:(pt + 1) * 128],
                            idh_s[:GB, :GB])
                        nc.vector.tensor_mul(wm8[:, sub],
                                             bcast_dim(ps_wt[:, :], 2, 16),
                                             colm8_s[:, 0:GB, :])
                    for b in range(GB):
                        nc.tensor.matmul(
                            ps_ct, lhsT=wm8[:, :, b, 0:GB], rhs=pn[:, i, b, :, :],
                            start=(pr == 0 and b == 0),
                            stop=(pr == NPR - 1 and b == GB - 1),
                            perf_mode=DR)
            # the GRU serial chain is interleaved with the next group's
            # first two passP blocks so the tensor queue never runs dry;
            # tensor order: stage_c -> em_a(0) -> ct transposes + ghT ->
            # em_a(1) (covers the gate elementwise chain) -> wah -> sp1
            if g + 1 < NG:
                fetch_pn(g + 1, 0)
                em_a(g + 1, 0, scalar_copies=True)
            gru_a(g, ps_ct)
            gru_b(g)
            if g + 1 < NG:
                fetch_peC(NPC * (g + 1) + 3)
                em_a(g + 1, 1)
            gru_c(g)
            if g + 1 < NG:
                em_b_tanh(g + 1, 0)
                em_sp1(g + 1, 0)
                em_exp1(g + 1, 0)
                em_b_tanh(g + 1, 1)
            # aP1 normalize + store: off the critical path
            w1g = w1_t.pop(g)
            for h in range(2):
                hs = slice(h * LP // 2, (h + 1) * LP // 2)
                apc = apb.tile([GB, LP // 2], F32, tag="ap", name="ap1c")
                nc.vector.tensor_scalar(apc, w1g[:, hs], rz1_t[g], None,
                                        op0=OP.mult)
                nc.gpsimd.dma_start(out=out[0, rows, hs], in_=apc)

        # ---- Q phase        # ---- Q phase: question-aware initial state rQ, all 8 batches ----
        tqT_s = _single([128, NHT, BL * LQ], F16, "tqT_s")
        cb_s = _single([128, NHT], F32, "cb_s")
        esq = _single([BL, LQ], F32, "esq")
        zq = _single([BL, 1], F32, "zq")
        rzq = _single([BL, 1], F32, "rzq")
        a_s = _single([BL, LQ], F16, "a_s")
        atm_s = _single([LQ, BL, BL], F16, "atm_s")

        def q1():
            ps_qv = ppp.tile([128, NHT], F32, tag="acc", name="ps_qv")
            for ht in range(NHT):
                for kt in range(NHT):
                    nc.tensor.matmul(ps_qv[:, ht:ht + 1],
                                     lhsT=WQvT_s[:, kt, ht * 128:(ht + 1) * 128],
                                     rhs=VQrT_s[:, kt, :], start=kt == 0,
                                     stop=kt == NHT - 1)
            nc.vector.tensor_add(cb_s, ps_qv, cqb_s)
            for ht in range(NHT):
                ps_tq = ppp.tile([128, 512], F32, tag="acc", name="ps_tq")
                for kt in range(NKT):
                    nc.tensor.matmul(ps_tq,
                                     lhsT=WQuT_s[:, kt, ht * 128:(ht + 1) * 128],
                                     rhs=qeT_s[:, kt, :], start=kt == 0,
                                     stop=kt == NKT - 1)
                nc.scalar.activation(tqT_s[:, ht, :], ps_tq, AF.Tanh,
                                     bias=cb_s[:, ht:ht + 1], scale=1.0)

        def q2():
            ps_sq = rowps.tile([BL, LQ], F32, tag="row", name="ps_sq")
            for b in range(BL):
                for ht in range(NHT):
                    nc.tensor.matmul(ps_sq, lhsT=vt1m_s[:, ht, b, :],
                                     rhs=tqT_s[:, ht, b * LQ:(b + 1) * LQ],
                                     start=(b == 0 and ht == 0),
                                     stop=(b == BL - 1 and ht == NHT - 1))
            nc.scalar.activation(esq, ps_sq, AF.Exp, accum_out=zq)
            nc.vector.reciprocal(rzq, zq)
            nc.vector.tensor_scalar(a_s, esq, rzq, None, op0=OP.mult)

        def q3():
            ps_at = ppp.tile([LQ, BL], F16, tag="acc", name="ps_at")
            nc.tensor.transpose(ps_at, a_s, idh_s[:BL, :BL])
            nc.vector.tensor_mul(atm_s,
                                 bcast_dim(ps_at[:, :], 1, BL),
                                 colm_s[0:LQ, :, :])
            ps_rq = rowps.tile([BL, D], F32, tag="row", name="ps_rq")
            for b in range(BL):
                nc.tensor.matmul(ps_rq, lhsT=atm_s[:, b, :],
                                 rhs=qeN_s[:, b * D:(b + 1) * D],
                                 start=b == 0, stop=b == BL - 1)
            nc.vector.tensor_copy(rq1_s, ps_rq)

        def q4():
            for kt in range(NKT):
                ps_t = ppp.tile([128, BL], F32, tag="acc", name="ps_q4")
                nc.tensor.transpose(ps_t, rq1_s[:, kt * 128:(kt + 1) * 128],
                                    idf_s[:BL, :BL])
                nc.vector.tensor_copy(rq1T_s[kt], ps_t)
                nc.vector.tensor_copy(rq1T8_s[kt // 2][:, kt % 2, 0:BL], ps_t)
            for ht in range(NHT):
                ps_w = ppp.tile([128, BL], F32, tag="acc", name="ps_w0")
                for kt in range(NKT):
                    nc.tensor.matmul(ps_w,
                                     lhsT=WahT_s[:, kt, ht * 128:(ht + 1) * 128],
                                     rhs=rq1T_s[kt], start=kt == 0,
                                     stop=kt == NKT - 1)
                nc.vector.tensor_scalar(biasP_s[:, 0, ht, :], ps_w,
                                        wb_s[:, ht, :], None, op0=OP.add)

        def emit_gi():
            # giT[gate, b] = (rq1 @ wih.T).T / 16 + bih, fp8 DoubleRow,
            # all 8 batches at once in transposed layout
            for gt in range(12):
                ps_gi = rowps.tile([128, BL], F32, tag="row", name="ps_gi")
                for pr in range(NKT // 2):
                    nc.tensor.matmul(
                        ps_gi, lhsT=wihT_s[:, pr, :, gt * 128:(gt + 1) * 128],
                        rhs=rq1T8_s[pr][:, :, 0:BL],
                        start=pr == 0, stop=pr == NKT // 2 - 1, perf_mode=DR)
                nc.vector.tensor_scalar(giT_s[:, gt, :], ps_gi, 1.0 / 16.0,
                                        None, op0=OP.mult)
            nc.vector.tensor_add(giT_s, giT_s, bihT_s)

        # ================= emission =================
        q1()
        fetch_peC(2)
        fetch_pn(0, 0)
        em_a(0, 0)
        q2()
        fetch_peC(3)
        fetch_pn(0, 1)
        em_a(0, 1)
        q3()
        fetch_peC(4)
        fetch_pn(0, 2)
        em_a(0, 2)
        q4()
        em_b_tanh(0, 0)
        fetch_peC(5)
        fetch_pn(0, 3)
        em_a(0, 3)
        em_sp1(0, 0)
        em_exp1(0, 0)
        em_b_tanh(0, 1)
        em_sp1(0, 1)
        em_exp1(0, 1)
        em_b_tanh(0, 2)
        em_sp1(0, 2)
        em_exp1(0, 2)
        em_b_tanh(0, 3)
        em_sp1(0, 3)
        em_exp1(0, 3)
        emit_gi()
        group_end(0)

        for g in range(1, NG):
            # P2(g-1, 0) mini-iteration; em_a/tanh1/sp1 for (g, 0..1) were
            # emitted inside group_end(g-1), interleaved with the GRU
            fetch_pn(g, 1)
            em_p2_tanh(g - 1, 0)
            em_p2_mm(g - 1, 0)
            em_p2_exp(g - 1, 0)
            for pc in (2, 3):
                c = NPC * g + pc
                if c + 2 < NG * NPC:
                    fetch_peC(c + 2)
                fetch_pn(g, pc)
                em_sp1(g, pc - 1)
                em_exp1(g, pc - 1)
                em_a(g, pc)
                em_p2_tanh(g - 1, pc - 1)
                em_p2_mm(g - 1, pc - 1)
                em_b_tanh(g, pc)
                em_p2_exp(g - 1, pc - 1)
            em_sp1(g, NPC - 1)
            em_exp1(g, NPC - 1)
            em_p2_tanh(g - 1, NPC - 1)
            em_p2_mm(g - 1, NPC - 1)
            em_p2_exp(g - 1, NPC - 1)
            finish_p2(g - 1)
            group_end(g)

        # tail: last group's pointer step 2
        for pc in range(NPC):
            em_p2_tanh(NG - 1, pc)
            em_p2_mm(NG - 1, pc)
            em_p2_exp(NG - 1, pc)
        finish_p2(NG - 1)

        zlp.release()
        w2p.release()
        w1p.release()
        ctps.release()
        rowps.release()
        ppp.release()
        grup.release()
        wmp.release()
        apb.release()
        t2bp.release()
        t2p.release()
        pnp.release()
        chunkp.release()
        sing.release()

    nc.compile()
    return nc


def _get_nc():
    global _CACHED_NC
    if _CACHED_NC is None:
        _CACHED_NC = _build()
    return _CACHED_NC


def _tiles(mat, nkt):  # [nkt*128, X] -> [128, nkt*X]
    x = mat.shape[1]
    return np.ascontiguousarray(
        mat.reshape(nkt, 128, x).transpose(1, 0, 2).reshape(128, nkt * x))


def _packA(f, Vt1, Vt2):
    # Vt1, Vt2: [BL, H] for this core's batch slice
    wp = np.zeros((128, WATOT), dtype=np.float16)

    def put(name, arr):
        o, ln = WA[name]
        assert arr.shape[1] == ln, (name, arr.shape, ln)
        wp[:arr.shape[0], o:o + ln] = arr

    put("WQvT", _tiles(f["WQv_W"].T.astype(np.float16), NHT))
    put("WQuT", _tiles(f["WQu_W"].T.astype(np.float16), NKT))
    put("WPhT", _tiles(f["WPh_W"].T.astype(np.float16), NKT))
    put("WahT", _tiles(f["Wah_W"].T.astype(np.float16), NKT))
    put("VQrT", _tiles(f["VQr"].reshape(1, H).T.astype(np.float16), NHT))
    # vt1m [128, ht, b, col]: col b = Vt1[b] per ht tile, rest zero
    v1 = np.zeros((128, NHT, BL, BL), dtype=np.float16)
    for b in range(BL):
        v1[:, :, b, b] = Vt1[b].reshape(NHT, 128).T
    put("vt1m", v1.reshape(128, NHT * BL * BL))
    # vt2g [128, ht, b, col]: col (b % GB) = Vt2[b], rest zero
    v2 = np.zeros((128, NHT, BL, GB), dtype=np.float16)
    for b in range(BL):
        v2[:, :, b, b % GB] = Vt2[b].reshape(NHT, 128).T
    put("vt2g", v2.reshape(128, NHT * BL * GB))
    put("idh", np.eye(128, dtype=np.float16))
    put("colm", np.broadcast_to(np.eye(BL, dtype=np.float16).reshape(1, BL * BL),
                                (128, BL * BL)))
    cm16 = np.hstack([np.eye(BL, dtype=np.float16),
                      np.zeros((BL, 16 - BL), dtype=np.float16)])
    put("cm16", np.broadcast_to(cm16.reshape(1, BL * 16), (128, BL * 16)))
    return wp


def _packG(f):
    # transposed gate biases: bhh_n as per-partition scalars [128, 4] for
    # the n-gate rows; bihT [128, 12, BL] with bhh pre-folded into the
    # r/z rows (rzin = gi + gh + bih + bhh, so the split is free)
    wp = np.zeros((128, 4 + 12 * BL), dtype=np.float16)
    bhh = (f["gru_bhh"].astype(np.float32)).reshape(12, 128).T
    bih = (f["gru_bih"].astype(np.float32)).reshape(12, 128).T
    wp[:, 0:4] = bhh[:, 8:12].astype(np.float16)
    comb = bih.copy()
    comb[:, 0:8] += bhh[:, 0:8]
    wp[:, 4:] = np.repeat(
        comb.astype(np.float16)[:, :, None], BL, axis=2).reshape(128, -1)
    return wp


def _packQ(qe):
    wp = np.zeros((128, WQTOT), dtype=np.float16)
    o, ln = WQ["qeT"]
    qeT = np.ascontiguousarray(qe.transpose(2, 1, 0)).astype(np.float16)
    wp[:, o:o + ln] = _tiles(qeT.reshape(D, BL * LQ), NKT)
    return wp


def _packB(f):
    # x16 lifts the ~N(0, 0.05^2) weights out of fp8's subnormal range;
    # compensated on-chip (gi: x1/16 in the bias add; gh: cancels the
    # ct fp8 copy's 1/16 pre-scale)
    wp = np.zeros((128, WBTOT), dtype=np.float32)
    o, ln = WB["wihT"]
    wp[:, o:o + ln] = _tiles(f["gru_wih"].T.astype(np.float32) * 16.0, NKT)
    o, ln = WB["whhT"]
    wp[:, o:o + ln] = _tiles(f["gru_whh"].T.astype(np.float32) * 16.0, NKT)
    return _fp8(wp)


def _pack32(f):
    wp = np.zeros((128, W32TOT), dtype=np.float32)
    o, ln = W32["idf"]
    wp[:, o:o + ln] = np.eye(128, dtype=np.float32)
    o, ln = W32["cqb"]
    wp[:, o:o + ln] = (f["WQu_b"] + f["WQv_b"]).astype(np.float32).reshape(NHT, 128).T
    o, ln = W32["wb"]
    wp[:, o:o + ln] = (f["WPh_b"] + f["Wah_b"]).astype(np.float32).reshape(NHT, 128).T
    o, ln = W32["bhhn"]
    wp[:, o:o + ln] = f["gru_bhh"].astype(np.float32).reshape(12, 128).T[:, 8:12]
    return wp


def _fp8(x):
    import ml_dtypes
    return np.ascontiguousarray(x).astype(ml_dtypes.float8_e4m3).view(np.uint8)


def make_in_maps(f):
    passEnc, quesEnc = f["passEnc"], f["quesEnc"]
    wp32 = _pack32(f)
    wpB = _packB(f)
    in_maps = []
    for i in range(NC):
        s = slice(i * BL, (i + 1) * BL)
        pe = passEnc[:, s, :]
        qe = quesEnc[:, s, :]
        wpAfull = _packA(f, f["Vt1"][s, :, 0], f["Vt2"][s, :, 0])
        wpQ_ = _packQ(qe)
        # peC [g, pc, part, b', kt, d]: per-partition runs of 8KB
        peC = np.ascontiguousarray(
            pe.astype(np.float16).reshape(NPC, 512, NG, GB, NKT, 128).transpose(
                2, 0, 5, 3, 4, 1))
        # peN8 [g, j, part, i, b', sub, d]: global p = (2j+i)*256 + sub*128
        # + part; per-partition contiguous runs of 4KB
        peN8 = _fp8(pe.reshape(NPR // 2, 2, 2, 128, NG, GB, D).transpose(
            4, 0, 3, 1, 5, 2, 6))
        in_maps.append({
            "peC": peC,
            "peN8": peN8,
            "wpA": np.ascontiguousarray(wpAfull[:, :WAHOT]),
            "wpA2": np.ascontiguousarray(wpAfull[:, WAHOT:]),
            "wpQ": wpQ_, "wpB": wpB, "wp32": wp32,
            "wpN": qe.astype(np.float16).reshape(LQ, BL * D),
            "wpG": _packG(f),
        })
    return in_maps


def kernel(**inputs):
    f = {k: np.asarray(v) for k, v in inputs.items()}
    in_maps = make_in_maps(f)
    nc = _get_nc()
    res = run_bass_kernel_spmd(nc, in_maps, core_ids=list(range(NC)))
    aP1 = np.concatenate([res.results[i]["out"][0] for i in range(NC)], axis=0)
    aP2 = np.concatenate([res.results[i]["out"][1] for i in range(NC)], axis=0)
    return (aP1.astype(np.float32), aP2.astype(np.float32))
